# revision 1
# baseline (speedup 1.0000x reference)
import sys

sys.path.insert(0, "/opt/trn_rl_repo")

import numpy as np

# ---- problem constants (hardcoded; kernel.py must be self-contained) ----
B, C, O, KK, H, W = 32, 64, 64, 3, 128, 128
COND = 16
NCORES = 8
BPC = B // NCORES          # samples per core = 4
NPAIR = BPC // 2           # sample-pairs per core = 2
HW = H * W                 # 16384
NT = H // 4                # 32 conv tiles (4 rows x 128 cols) per pair
XOFF = 128                 # elem offset of the padded-x region in the work buffer
WP = W + 2                 # 130: padded row width (zero col left/right)
WORK_ELEMS = XOFF + WP * (H + 2)  # out_pre [0,16384) + padded x
GEN_W = O * C + C * KK * KK + O   # 4096 + 576 + 64 = 4736 generator outputs
NVALID = 63 * 63           # VALID conv output positions of the cond conv
EPS = 1e-5

# matmul dtype for the heavy convs: float32r streams at 1 col/cycle (vs 4 for
# plain float32) while carrying fp32 bits. Flip to "float32" if f32r numerics
# are off on hardware.
MM_DT_NAME = "float32r"



# ---------------------------------------------------------------------------
# host-side constant prep (numpy only)
# ---------------------------------------------------------------------------
def _prep_consts(inp):
    f32 = np.float32
    cg_w1 = np.asarray(inp["cg_w1"], f32)      # [16, 64, 3, 3]
    cg_b1 = np.asarray(inp["cg_b1"], f32)      # [16]
    cg_w2 = np.asarray(inp["cg_w2"], f32)      # [16, 16]
    cg_b2 = np.asarray(inp["cg_b2"], f32)      # [16]
    wg_w = np.asarray(inp["wg_w"], f32)        # [576, 16]
    wg_b = np.asarray(inp["wg_b"], f32)        # [576]
    pg_w = np.asarray(inp["pg_w"], f32)        # [4096, 16]
    pg_b = np.asarray(inp["pg_b"], f32)        # [4096]
    bg_w = np.asarray(inp["bg_w"], f32)        # [64, 16]
    bg_b = np.asarray(inp["bg_b"], f32)        # [64]

    # cond-conv taps folded with the 1/3969 spatial mean:
    # w1taps[s*64+ci, 32*k + s*16+co] = cg_w1[co, ci, ky, kx] / 3969
    w1taps = np.zeros((128, 9 * 32), f32)
    for k in range(9):
        ky, kx = k // 3, k % 3
        blk = (cg_w1[:, :, ky, kx] / NVALID).T  # [ci, co]
        for s in range(2):
            w1taps[s * 64:(s + 1) * 64, 32 * k + s * 16: 32 * k + s * 16 + 16] = blk

    b1x2 = np.concatenate([cg_b1, cg_b1]).reshape(32, 1)
    b2x2 = np.concatenate([cg_b2, cg_b2]).reshape(32, 1)

    cw2 = np.zeros((32, 32), f32)
    for s in range(2):
        cw2[s * 16:(s + 1) * 16, s * 16:(s + 1) * 16] = cg_w2.T  # [ci, co]

    # generator moving operand: rows 0-15 and 16-31 both hold G^T, row 32 bias.
    # pw block stored c-major (flat index c*64+o) so the later SBUF rearrange
    # DMA has a contiguous inner dim.
    pg_w_co = pg_w.reshape(O, C, COND).transpose(1, 0, 2).reshape(O * C, COND)
    pg_b_co = pg_b.reshape(O, C).T.reshape(-1)
    G = np.concatenate([pg_w_co, wg_w, bg_w], axis=0)            # [4736, 16]
    gbias = np.concatenate([pg_b_co, wg_b, bg_b])                # [4736]
    rhs_gen = np.zeros((33, GEN_W), f32)
    rhs_gen[0:16] = G.T
    rhs_gen[16:32] = G.T
    rhs_gen[32] = gbias

    gammab = np.asarray(inp["bn_gamma"], f32).reshape(64, 1)
    betab = np.asarray(inp["bn_beta"], f32).reshape(64, 1)

    zeros_st = np.zeros((128, 9 * 128), f32)
    sgen_init = np.zeros((33, 2), f32)
    sgen_init[32, :] = 1.0

    return {
        "w1taps": w1taps, "b1x2": b1x2, "cw2": cw2, "b2x2": b2x2,
        "rhs_gen": rhs_gen, "gammab": gammab, "betab": betab,
        "zeros_st": zeros_st, "sgen_init": sgen_init,
    }


# ---------------------------------------------------------------------------
# kernel body: emits one core's program under TileContext
#   ins:  dict of DRAM APs {x, w1taps, b1x2, cw2, b2x2, rhs_gen, gammab, betab}
#   outs: dict {y}
# ---------------------------------------------------------------------------
def body(tc, outs, ins):
    import concourse.bass as bass
    from concourse import mybir

    nc = tc.nc
    f32 = mybir.dt.float32
    mmdt = getattr(mybir.dt, MM_DT_NAME)
    AX = mybir.AxisListType
    ALU = mybir.AluOpType
    ACT = mybir.ActivationFunctionType

    x_d = ins["x"].rearrange("b c h w -> b c (h w)")      # [4, 64, 16900] padded
    y_d = outs["y"].rearrange("b c h w -> b c (h w)")     # [4, 64, 16384]

    with (
        tc.tile_pool(name="work", bufs=1) as work_pool,
        tc.tile_pool(name="consts", bufs=1) as cpool,
        tc.tile_pool(name="pairbuf", bufs=2) as ppool,
        tc.tile_pool(name="sq", bufs=2) as sqpool,
        tc.tile_pool(name="stats", bufs=1) as stpool,
        tc.tile_pool(name="cpsum", bufs=7, space="PSUM") as cpsum,
        tc.tile_pool(name="gpsum", bufs=1, space="PSUM") as gpsum,
        tc.tile_pool(name="dram", bufs=1, space="DRAM") as dpool,
    ):
        # ---- constants into SBUF ----
        w1taps = cpool.tile([128, 9 * 32], f32, tag="w1taps", name="w1taps")
        b1x2 = cpool.tile([32, 1], f32, tag="b1x2", name="b1x2")
        cw2 = cpool.tile([32, 32], f32, tag="cw2", name="cw2")
        b2x2 = cpool.tile([32, 1], f32, tag="b2x2", name="b2x2")
        rhs_gen = cpool.tile([33, GEN_W], mmdt, tag="rhs_gen", name="rhs_gen")
        gammab = cpool.tile([64, 1], f32, tag="gammab", name="gammab")
        betab = cpool.tile([64, 1], f32, tag="betab", name="betab")
        for t_, n_ in ((w1taps, "w1taps"), (b1x2, "b1x2"), (cw2, "cw2"),
                       (b2x2, "b2x2"), (gammab, "gammab"), (betab, "betab")):
            nc.sync.dma_start(t_[:], ins[n_])
        nc.sync.dma_start(rhs_gen[:], ins["rhs_gen"].bitcast(mmdt))

        # ---- persistent state ----
        works = [work_pool.tile([128, WORK_ELEMS], f32, tag=f"work{p}", name=f"work{p}")
                 for p in range(NPAIR)]
        dbias2 = stpool.tile([128, NPAIR], f32, tag="dbias2", name="dbias2")
        sums = stpool.tile([128, NPAIR * NT], f32, tag="sums", name="sums")
        sumsqs = stpool.tile([128, NPAIR * NT], f32, tag="sumsqs", name="sumsqs")
        s_tiles = [None] * NPAIR

        def xview(wp):
            # padded x: row h of the image lives at xv[:, h+1, 1:129]
            return wp[:, XOFF:XOFF + WP * (H + 2)].rearrange(
                "p (r w) -> p r w", r=H + 2, w=WP)

        ROWSEG = [0, 33, 66, 99, H + 2]

        def load_pair(p):
            wp = works[p]
            # x arrives pre-padded (130x130 with zero border) from the host;
            # one DMA per row segment covers both samples -> 128 partitions
            for ch in range(4):
                e0, e1 = WP * ROWSEG[ch], WP * ROWSEG[ch + 1]
                nc.sync.dma_start(
                    wp[:, XOFF + e0: XOFF + e1].bitcast(mmdt),
                    x_d[2 * p:2 * p + 2, :, e0:e1].bitcast(mmdt),
                )

        def prep_pair(p):
            """cond generator -> dynamic weights -> S_k stationaries for pair p."""
            wp = works[p]
            xv = xview(wp)

            # E/O row sums (even / odd columns) over the 128 image rows
            E = ppool.tile([128, H], f32, tag="E", name="E", bufs=1)
            Od = ppool.tile([128, H], f32, tag="Od", name="Od", bufs=1)
            for ch in range(4):
                r0 = 1 + 32 * ch
                nc.vector.tensor_reduce(
                    E[:, 32 * ch: 32 * ch + 32],
                    xv[:, r0:r0 + 32, 1:128:2], axis=AX.X, op=ALU.add)
                nc.vector.tensor_reduce(
                    Od[:, 32 * ch: 32 * ch + 32],
                    xv[:, r0:r0 + 32, 2:129:2], axis=AX.X, op=ALU.add)
            # per-row strided col sums for kx = 0,1,2
            R = ppool.tile([128, 3 * H], f32, tag="R", name="R", bufs=1)
            colv = xv[:, 1:1 + H, :]
            nc.vector.tensor_tensor(
                out=R[:, 0:H], in0=E[:], in1=colv[:, :, 127:128], op=ALU.subtract)
            nc.vector.tensor_tensor(
                out=R[:, H:2 * H], in0=Od[:], in1=colv[:, :, 128:129], op=ALU.subtract)
            nc.vector.tensor_tensor(
                out=R[:, 2 * H:3 * H], in0=E[:], in1=colv[:, :, 1:2], op=ALU.subtract)
            # T[ci, k] strided row sums
            Tt = ppool.tile([128, 9], f32, tag="Tt", name="Tt", bufs=1)
            for k in range(9):
                ky, kx = k // 3, k % 3
                nc.vector.tensor_reduce(
                    Tt[:, k:k + 1],
                    R[:, kx * H + ky: kx * H + ky + 125: 2],  # 63 rows
                    axis=AX.X, op=ALU.add)

            # cond chain (tiny matmuls, plain fp32)
            pc1 = gpsum.tile([128, 512], f32, tag="gp", name="gp")
            for i, k in enumerate(range(9)):
                nc.tensor.matmul(
                    pc1[0:32, 0:1], w1taps[:, 32 * k: 32 * k + 32],
                    Tt[:, k:k + 1], start=(i == 0), stop=(i == 8))
            cond1 = ppool.tile([32, 1], f32, tag="cond1", name="cond1", bufs=1)
            nc.scalar.activation(cond1[:], pc1[0:32, 0:1], ACT.Relu, bias=b1x2[:])
            pc2 = gpsum.tile([128, 512], f32, tag="gp", name="gp")
            nc.tensor.matmul(pc2[0:32, 0:1], cw2[:], cond1[:])
            cond2 = ppool.tile([32, 1], f32, tag="cond2", name="cond2", bufs=1)
            nc.scalar.activation(cond2[:], pc2[0:32, 0:1], ACT.Relu, bias=b2x2[:])

            # generator stationary [33, 2]: col s = cond2_s (rows 16s..16s+16), row 32 = 1
            sgen = ppool.tile([33, 2], mmdt, tag="sgen", name="sgen", bufs=1)
            nc.sync.dma_start(sgen[:], ins["sgen_init"].bitcast(mmdt))
            nc.sync.dma_start(sgen[0:16, 0:1], cond2[0:16, :].bitcast(mmdt))
            nc.sync.dma_start(sgen[16:32, 1:2], cond2[16:32, :].bitcast(mmdt))

            # generator matmuls -> gen_sb [2, 4736] (pw | dw | dbias), relu'd
            gen_sb = ppool.tile([2, GEN_W], f32, tag="gen_sb", name="gen_sb", bufs=1)
            for i in range(10):
                c0 = 512 * i
                n = min(512, GEN_W - c0)
                gp = gpsum.tile([128, 512], f32, tag="gp", name="gp")
                nc.tensor.matmul(
                    gp[0:2, 0:n], sgen[:], rhs_gen[:, c0:c0 + n])
                if c0 >= 4608:  # last chunk: dw tail (relu) + dbias (no relu)
                    nc.scalar.activation(gen_sb[0:2, 4608:4672], gp[0:2, 0:64], ACT.Relu)
                    nc.scalar.copy(gen_sb[0:2, 4672:4736], gp[0:2, 64:128])
                else:
                    nc.scalar.activation(gen_sb[0:2, c0:c0 + n], gp[0:2, 0:n], ACT.Relu)

            # rearrange to channel-major layouts
            pwcb = ppool.tile([128, O], f32, tag="pwcb", name="pwcb", bufs=1)
            dwcb = ppool.tile([128, 9], f32, tag="dwcb", name="dwcb", bufs=1)
            for s in range(2):
                nc.sync.dma_start(
                    pwcb[s * 64:(s + 1) * 64, :],
                    gen_sb[s:s + 1, 0:O * C].rearrange(
                        "s (c o) -> s c o", o=O, c=C))
                nc.sync.dma_start(
                    dwcb[s * 64:(s + 1) * 64, :],
                    gen_sb[s:s + 1, O * C:O * C + C * 9].rearrange(
                        "s (c k) -> s c k", c=C, k=9))
                nc.sync.dma_start(dbias2[s * 64:(s + 1) * 64, p:p + 1],
                                  gen_sb[s:s + 1, 4672:4736])

            # S_k stationaries: S[s*64+c, 128k + s*64+o] = pw[o,c]*dw[c,k]
            st = ppool.tile([128, 9 * 128], mmdt, tag="stat_w", name="stat_w", bufs=2)
            nc.sync.dma_start(st[:], ins["zeros_st"].bitcast(mmdt))
            for k in range(9):
                for s in range(2):
                    nc.vector.tensor_scalar(
                        out=st[s * 64:(s + 1) * 64,
                               128 * k + s * 64: 128 * k + s * 64 + 64],
                        in0=pwcb[s * 64:(s + 1) * 64, :],
                        scalar1=dwcb[s * 64:(s + 1) * 64, k:k + 1],
                        scalar2=None, op0=ALU.mult)  # out rounds to f32r
            s_tiles[p] = st

        def conv_group(p, g):
            """4 conv tiles (16 output rows) for pair p, group g."""
            wp = works[p]
            xv = xview(wp)
            st = s_tiles[p]
            tiles = range(4 * g, 4 * g + 4)
            psums = {}
            for t in tiles:
                psums[t] = cpsum.tile([128, 512], f32, tag="cp", name="cp")
            for i in range(9):
                ky, kx = i // 3, i % 3
                lhsT = st[:, 128 * i: 128 * i + 128]
                for t in tiles:
                    h0 = 4 * t
                    # out rows h0..h0+3, tap (ky,kx): x rows h0+ky-1.., cols +kx-1
                    nc.tensor.matmul(
                        psums[t][:],
                        lhsT,
                        xv[:, h0 + ky: h0 + ky + 4, kx:kx + 128].bitcast(mmdt),
                        start=(i == 0), stop=(i == 8))
            for t in tiles:
                col = NT * p + t
                # copy pre-BN conv tile into the shared buffer + per-partition sum
                nc.vector.tensor_scalar(
                    out=wp[:, 512 * t: 512 * t + 512].bitcast(mmdt),
                    in0=psums[t][:], scalar1=0.0, scalar2=0.0, op0=ALU.add,
                    op1=ALU.add, accum_out=sums[:, col:col + 1])
                sq = sqpool.tile([128, 512], f32, tag="sq", name="sq")
                nc.scalar.activation(
                    sq[:], psums[t][:], ACT.Square,
                    accum_out=sumsqs[:, col:col + 1])

        # per-pair stats fixup, overlapped with the other pair's conv:
        # produces [128, 2] (sum, sumsq) with the dbias fold applied
        sum2 = stpool.tile([128, NPAIR], f32, tag="sum2", name="sum2")
        ssq2 = stpool.tile([128, NPAIR], f32, tag="ssq2", name="ssq2")

        def pair_stats(p):
            pc = p * NT
            nc.vector.tensor_reduce(
                sum2[:, p:p + 1], sums[:, pc:pc + NT], axis=AX.X, op=ALU.add)
            nc.vector.tensor_reduce(
                ssq2[:, p:p + 1], sumsqs[:, pc:pc + NT], axis=AX.X, op=ALU.add)
            d16k = stpool.tile([128, 1], f32, tag="d16k", name="d16k", bufs=2)
            nc.vector.tensor_scalar(out=d16k[:], in0=dbias2[:, p:p + 1],
                                    scalar1=float(HW), scalar2=None, op0=ALU.mult)
            t1 = stpool.tile([128, 1], f32, tag="t1", name="t1", bufs=2)
            # t1 = 2*d*sum + n*d^2 = d*(2*sum + n*d)
            nc.vector.tensor_scalar(out=t1[:], in0=sum2[:, p:p + 1], scalar1=2.0,
                                    scalar2=None, op0=ALU.mult)
            nc.vector.tensor_tensor(out=t1[:], in0=t1[:], in1=d16k[:], op=ALU.add)
            nc.vector.tensor_tensor(out=t1[:], in0=t1[:], in1=dbias2[:, p:p + 1],
                                    op=ALU.mult)
            nc.vector.tensor_tensor(out=ssq2[:, p:p + 1], in0=ssq2[:, p:p + 1],
                                    in1=t1[:], op=ALU.add)
            nc.vector.tensor_tensor(out=sum2[:, p:p + 1], in0=sum2[:, p:p + 1],
                                    in1=d16k[:], op=ALU.add)

        # ---------------- main schedule ----------------
        load_pair(0)
        prep_pair(0)
        for g in range(5):
            conv_group(0, g)
        load_pair(1)
        prep_pair(1)
        for g in range(5, 8):
            conv_group(0, g)
        pair_stats(0)
        for g in range(8):
            conv_group(1, g)
        pair_stats(1)

        # ---------------- BN statistics ----------------
        st128 = stpool.tile([128, 2], f32, tag="st128", name="st128")
        nc.vector.tensor_reduce(st128[:, 0:1], sum2[:], axis=AX.X, op=ALU.add)
        nc.vector.tensor_reduce(st128[:, 1:2], ssq2[:], axis=AX.X, op=ALU.add)
        # fold the two sample-halves: [128, 2] -> [64, 2]
        stlo = stpool.tile([64, 2], f32, tag="stlo", name="stlo")
        nc.sync.dma_start(stlo[:], st128[64:128, :])
        st64 = stpool.tile([64, 2], f32, tag="st64", name="st64")
        nc.vector.tensor_tensor(out=st64[:], in0=st128[0:64, :], in1=stlo[:],
                                op=ALU.add)

        # all-reduce across the 8 cores via DRAM bounce buffers
        cc_in = dpool.tile([64, 2], f32, tag="cc_in", name="cc_in")
        cc_out = dpool.tile([64, 2], f32, tag="cc_out", name="cc_out")
        nc.sync.dma_start(cc_in[:], st64[:])
        nc.gpsimd.collective_compute(
            "AllReduce", ALU.add,
            replica_groups=[list(range(NCORES))],
            ins=[cc_in[:].opt()], outs=[cc_out[:].opt()])
        stg = stpool.tile([64, 2], f32, tag="stg", name="stg")
        nc.sync.dma_start(stg[:], cc_out[:])

        # scale/shift: S = gamma/sqrt(var+eps) ; T2[:,p] = dbias*S + (beta - mean*S)
        ntot = float(BPC * NCORES * HW)
        mean = stpool.tile([64, 1], f32, tag="mean", name="mean")
        nc.vector.tensor_scalar(out=mean[:], in0=stg[:, 0:1], scalar1=1.0 / ntot,
                                scalar2=None, op0=ALU.mult)
        var = stpool.tile([64, 1], f32, tag="var", name="var")
        nc.vector.tensor_scalar(out=var[:], in0=stg[:, 1:2], scalar1=1.0 / ntot,
                                scalar2=None, op0=ALU.mult)
        msq = stpool.tile([64, 1], f32, tag="msq", name="msq")
        nc.vector.tensor_tensor(out=msq[:], in0=mean[:], in1=mean[:], op=ALU.mult)
        nc.vector.tensor_tensor(out=var[:], in0=var[:], in1=msq[:], op=ALU.subtract)
        nc.vector.tensor_scalar(out=var[:], in0=var[:], scalar1=EPS,
                                scalar2=None, op0=ALU.add)
        std = stpool.tile([64, 1], f32, tag="std", name="std")
        nc.scalar.activation(std[:], var[:], ACT.Sqrt)
        inv = stpool.tile([64, 1], f32, tag="inv", name="inv")
        nc.vector.reciprocal(inv[:], std[:])
        stpk = stpool.tile([64, 2], f32, tag="stpk", name="stpk")
        nc.vector.tensor_tensor(out=stpk[:, 0:1], in0=inv[:], in1=gammab[:],
                                op=ALU.mult)
        nc.vector.tensor_tensor(out=stpk[:, 1:2], in0=mean[:], in1=stpk[:, 0:1],
                                op=ALU.mult)
        nc.vector.tensor_tensor(out=stpk[:, 1:2], in0=betab[:], in1=stpk[:, 1:2],
                                op=ALU.subtract)
        STb = stpool.tile([128, 2], f32, tag="STb", name="STb")
        nc.vector.tensor_copy(STb[0:64, :], stpk[:])
        nc.sync.dma_start(STb[64:128, :], stpk[:])
        Sb = STb
        T2 = stpool.tile([128, NPAIR], f32, tag="T2", name="T2")
        nc.vector.tensor_scalar(out=T2[:], in0=dbias2[:], scalar1=STb[:, 0:1],
                                scalar2=STb[:, 1:2], op0=ALU.mult, op1=ALU.add)

        # ---------------- final affine + store ----------------
        CH = 2048
        for p in range(NPAIR):
            wp = works[p]
            for i in range(HW // CH):
                c0 = CH * i
                nc.vector.tensor_scalar(
                    out=wp[:, c0:c0 + CH].bitcast(mmdt), in0=wp[:, c0:c0 + CH],
                    scalar1=Sb[:, 0:1], scalar2=T2[:, p:p + 1],
                    op0=ALU.mult, op1=ALU.add)
                nc.sync.dma_start(
                    y_d[2 * p:2 * p + 2, :, c0:c0 + CH], wp[:, c0:c0 + CH])


# ---------------------------------------------------------------------------
# build + run
# ---------------------------------------------------------------------------
_CACHE = {}


def _build():
    if "nc" in _CACHE:
        return _CACHE["nc"]
    from concourse import bacc, mybir, tile

    nc = bacc.Bacc("TRN2", target_bir_lowering=False, debug=False,
                   num_devices=NCORES)
    f32 = mybir.dt.float32
    ins = {
        "x": nc.dram_tensor("x", [BPC, C, H + 2, W + 2], f32, kind="ExternalInput").ap(),
        "w1taps": nc.dram_tensor("w1taps", [128, 9 * 32], f32, kind="ExternalInput").ap(),
        "b1x2": nc.dram_tensor("b1x2", [32, 1], f32, kind="ExternalInput").ap(),
        "cw2": nc.dram_tensor("cw2", [32, 32], f32, kind="ExternalInput").ap(),
        "b2x2": nc.dram_tensor("b2x2", [32, 1], f32, kind="ExternalInput").ap(),
        "rhs_gen": nc.dram_tensor("rhs_gen", [33, GEN_W], f32, kind="ExternalInput").ap(),
        "gammab": nc.dram_tensor("gammab", [64, 1], f32, kind="ExternalInput").ap(),
        "betab": nc.dram_tensor("betab", [64, 1], f32, kind="ExternalInput").ap(),
        "zeros_st": nc.dram_tensor("zeros_st", [128, 9 * 128], f32,
                                   kind="ExternalInput").ap(),
        "sgen_init": nc.dram_tensor("sgen_init", [33, 2], f32,
                                    kind="ExternalInput").ap(),
    }
    outs = {"y": nc.dram_tensor("y", [BPC, C, H, W], f32, kind="ExternalOutput").ap()}
    with tile.TileContext(nc) as tc:
        body(tc, outs, ins)
    nc.compile()
    _CACHE["nc"] = nc
    return nc


def make_in_maps(inputs):
    x = np.asarray(inputs["x"], np.float32)
    xp = np.zeros((B, C, H + 2, W + 2), np.float32)
    xp[:, :, 1:H + 1, 1:W + 1] = x
    consts = _prep_consts(inputs)
    in_maps = []
    for c in range(NCORES):
        m = {"x": np.ascontiguousarray(xp[BPC * c: BPC * (c + 1)])}
        m.update(consts)
        in_maps.append(m)
    return in_maps


def run(inputs, trace=False):
    from concourse.bass_utils import run_bass_kernel_spmd

    nc = _build()
    in_maps = make_in_maps(inputs)
    res = run_bass_kernel_spmd(nc, in_maps, core_ids=list(range(NCORES)),
                               trace=trace)
    y = np.concatenate([res.results[c]["y"] for c in range(NCORES)], axis=0)
    return y, res


def kernel(**inputs) -> np.ndarray:
    y, _ = run(inputs, trace=False)
    return y



# revision 7
# speedup vs baseline: 1.1769x; 1.1769x over previous
import sys

sys.path.insert(0, "/opt/trn_rl_repo")

import numpy as np
import ml_dtypes

BF16 = ml_dtypes.bfloat16

# ---- problem constants (hardcoded; kernel.py must be self-contained) ----
B, C, O, KK, H, W = 32, 64, 64, 3, 128, 128
COND = 16
NCORES = 8
BPC = B // NCORES          # samples per core = 4
NPAIR = BPC // 2           # sample-pairs per core = 2
HW = H * W                 # 16384
NT = H // 4                # 32 conv tiles (4 rows x 128 cols) per pair
WP = W + 2                 # 130: padded row width (zero col left/right)
XELEMS = WP * (H + 2)      # padded x elems per channel
GEN_W = O * C + C * KK * KK + O   # 4096 + 576 + 64 = 4736 generator outputs
NVALID = 63 * 63           # VALID conv output positions of the cond conv
EPS = 1e-5

# pad-row segments, aligned so E/Od chunk ch only needs segment ch
ROWSEG = [0, 33, 65, 97, H + 2]


# ---------------------------------------------------------------------------
# host-side constant prep (numpy only)
# ---------------------------------------------------------------------------
def _prep_consts(inp):
    f32 = np.float32
    cg_w1 = np.asarray(inp["cg_w1"], f32)      # [16, 64, 3, 3]
    cg_b1 = np.asarray(inp["cg_b1"], f32)      # [16]
    cg_w2 = np.asarray(inp["cg_w2"], f32)      # [16, 16]
    cg_b2 = np.asarray(inp["cg_b2"], f32)      # [16]
    wg_w = np.asarray(inp["wg_w"], f32)        # [576, 16]
    wg_b = np.asarray(inp["wg_b"], f32)        # [576]
    pg_w = np.asarray(inp["pg_w"], f32)        # [4096, 16]
    pg_b = np.asarray(inp["pg_b"], f32)        # [4096]
    bg_w = np.asarray(inp["bg_w"], f32)        # [64, 16]
    bg_b = np.asarray(inp["bg_b"], f32)        # [64]

    # cond-conv taps folded with the 1/3969 spatial mean:
    # w1taps[s*64+ci, 32*k + s*16+co] = cg_w1[co, ci, ky, kx] / 3969
    w1taps = np.zeros((128, 9 * 32), f32)
    for k in range(9):
        ky, kx = k // 3, k % 3
        blk = (cg_w1[:, :, ky, kx] / NVALID).T  # [ci, co]
        for s in range(2):
            w1taps[s * 64:(s + 1) * 64, 32 * k + s * 16: 32 * k + s * 16 + 16] = blk

    b1x2 = np.concatenate([cg_b1, cg_b1]).reshape(32, 1)
    b2x2 = np.concatenate([cg_b2, cg_b2]).reshape(32, 1)

    cw2 = np.zeros((32, 32), f32)
    for s in range(2):
        cw2[s * 16:(s + 1) * 16, s * 16:(s + 1) * 16] = cg_w2.T  # [ci, co]

    # generator moving operand: rows 0-15 and 16-31 both hold G^T, row 32 bias.
    # pw block stored c-major (flat index c*64+o) so the later SBUF rearrange
    # DMA has a contiguous inner dim.
    pg_w_co = pg_w.reshape(O, C, COND).transpose(1, 0, 2).reshape(O * C, COND)
    pg_b_co = pg_b.reshape(O, C).T.reshape(-1)
    G = np.concatenate([pg_w_co, wg_w, bg_w], axis=0)            # [4736, 16]
    gbias = np.concatenate([pg_b_co, wg_b, bg_b])                # [4736]
    rhs_gen = np.zeros((33, GEN_W), f32)
    rhs_gen[0:16] = G.T
    rhs_gen[16:32] = G.T
    rhs_gen[32] = gbias

    gammab = np.asarray(inp["bn_gamma"], f32).reshape(64, 1)
    betab = np.asarray(inp["bn_beta"], f32).reshape(64, 1)

    # maskc[i, s] = 1 if i // 16 == s  (for building the gen stationary)
    maskc = np.zeros((32, 2), f32)
    maskc[0:16, 0] = 1.0
    maskc[16:32, 1] = 1.0

    return {
        "w1taps": w1taps, "b1x2": b1x2, "cw2": cw2, "b2x2": b2x2,
        "rhs_gen": rhs_gen.astype(BF16), "gammab": gammab, "betab": betab,
        "maskc": maskc,
    }


# ---------------------------------------------------------------------------
# kernel body: emits one core's program under TileContext
# ---------------------------------------------------------------------------
def body(tc, outs, ins):
    import concourse.bass as bass
    from concourse import mybir

    nc = tc.nc
    f32 = mybir.dt.float32
    bf16 = mybir.dt.bfloat16
    AX = mybir.AxisListType
    ALU = mybir.AluOpType
    ACT = mybir.ActivationFunctionType

    x_d = ins["x"].rearrange("b c h w -> b c (h w)")      # [4, 64, 16900] bf16
    y_d = outs["y"].rearrange("b c h w -> b c (h w)")     # [4, 64, 16384] bf16

    with (
        tc.tile_pool(name="xbuf", bufs=1) as xpool,
        tc.tile_pool(name="obuf", bufs=1) as opool,
        tc.tile_pool(name="consts", bufs=1) as cpool,
        tc.tile_pool(name="pairbuf", bufs=2) as ppool,
        tc.tile_pool(name="sq", bufs=2) as sqpool,
        tc.tile_pool(name="stats", bufs=1) as stpool,
        tc.tile_pool(name="cpsum", bufs=6, space="PSUM") as cpsum,
        tc.tile_pool(name="gpsum", bufs=2, space="PSUM") as gpsum,
        tc.tile_pool(name="dram", bufs=1, space="DRAM") as dpool,
    ):
        # ---- constants into SBUF ----
        w1taps = cpool.tile([128, 9 * 32], f32, tag="w1taps", name="w1taps")
        b1x2 = cpool.tile([32, 1], f32, tag="b1x2", name="b1x2")
        cw2 = cpool.tile([32, 32], f32, tag="cw2", name="cw2")
        b2x2 = cpool.tile([32, 1], f32, tag="b2x2", name="b2x2")
        rhs_gen = cpool.tile([33, GEN_W], bf16, tag="rhs_gen", name="rhs_gen")
        gammab = cpool.tile([64, 1], f32, tag="gammab", name="gammab")
        betab = cpool.tile([64, 1], f32, tag="betab", name="betab")
        maskc = cpool.tile([32, 2], f32, tag="maskc", name="maskc")
        for t_, n_ in ((w1taps, "w1taps"), (b1x2, "b1x2"), (cw2, "cw2"),
                       (b2x2, "b2x2"), (gammab, "gammab"), (betab, "betab"),
                       (rhs_gen, "rhs_gen"), (maskc, "maskc")):
            nc.sync.dma_start(t_[:], ins[n_])

        # ---- persistent state ----
        xbs = [xpool.tile([128, XELEMS], bf16, tag=f"xb{p}", name=f"xb{p}")
               for p in range(NPAIR)]
        obs = [opool.tile([128, HW], bf16, tag=f"ob{p}", name=f"ob{p}")
               for p in range(NPAIR)]
        sts = [cpool.tile([128, 9 * 128], bf16, tag=f"st{p}", name=f"st{p}")
               for p in range(NPAIR)]
        sgens = [cpool.tile([33, 2], bf16, tag=f"sgen{p}", name=f"sgen{p}")
                 for p in range(NPAIR)]
        # zero the stationaries once (block structure is identical per pair);
        # set the all-ones bias row of the gen stationaries once
        for p in range(NPAIR):
            nc.gpsimd.memset(sts[p][:], 0.0)
            nc.gpsimd.memset(sgens[p][0:32, :], 0.0)
            nc.gpsimd.memset(sgens[p][32:33, :], 1.0)

        dbias2 = stpool.tile([128, NPAIR], bf16, tag="dbias2", name="dbias2")
        sums = stpool.tile([128, NPAIR * NT], f32, tag="sums", name="sums")
        sumsqs = stpool.tile([128, NPAIR * NT], f32, tag="sumsqs", name="sumsqs")

        def xview(p):
            # padded x: row h of the image lives at xv[:, h+1, 1:129]
            return xbs[p][:].rearrange("p (r w) -> p r w", r=H + 2, w=WP)

        def load_seg(p, ch):
            e0, e1 = WP * ROWSEG[ch], WP * ROWSEG[ch + 1]
            nc.sync.dma_start(xbs[p][:, e0:e1], x_d[2 * p:2 * p + 2, :, e0:e1])

        # E/Od row sums (even / odd columns); chunk ch covers image rows
        # 32ch..32ch+31 and only needs x segment ch
        def eod_chunk(p, E, Od, ch):
            xv = xview(p)
            r0 = 1 + 32 * ch
            nc.vector.tensor_reduce(
                E[:, 32 * ch: 32 * ch + 32],
                xv[:, r0:r0 + 32, 1:128:2], axis=AX.X, op=ALU.add)
            nc.vector.tensor_reduce(
                Od[:, 32 * ch: 32 * ch + 32],
                xv[:, r0:r0 + 32, 2:129:2], axis=AX.X, op=ALU.add)

        def prep_pair(p, E, Od):
            """cond generator -> dynamic weights -> S_k stationary for pair p.

            E/Od row sums must already be computed (pipelined with the load).
            """
            xv = xview(p)
            # border columns 1, 127, 128 as f32 for the strided col sums
            bord = ppool.tile([128, 3 * H], f32, tag="bord", name="bord")
            colv = xv[:, 1:1 + H, :]
            nc.vector.tensor_copy(bord[:].rearrange("p (c r) -> p c r", c=3, r=H)[:, 0, :],
                                  colv[:, :, 127])
            nc.vector.tensor_copy(bord[:].rearrange("p (c r) -> p c r", c=3, r=H)[:, 1, :],
                                  colv[:, :, 128])
            nc.vector.tensor_copy(bord[:].rearrange("p (c r) -> p c r", c=3, r=H)[:, 2, :],
                                  colv[:, :, 1])
            # per-row strided col sums for kx = 0,1,2
            R = ppool.tile([128, 3 * H], f32, tag="R", name="R")
            nc.vector.tensor_tensor(
                out=R[:, 0:H], in0=E[:], in1=bord[:, 0:H], op=ALU.subtract)
            nc.vector.tensor_tensor(
                out=R[:, H:2 * H], in0=Od[:], in1=bord[:, H:2 * H], op=ALU.subtract)
            nc.vector.tensor_tensor(
                out=R[:, 2 * H:3 * H], in0=E[:], in1=bord[:, 2 * H:3 * H],
                op=ALU.subtract)
            # T[ci, k] strided row sums
            Tt = ppool.tile([128, 9], f32, tag="Tt", name="Tt")
            for k in range(9):
                ky, kx = k // 3, k % 3
                nc.vector.tensor_reduce(
                    Tt[:, k:k + 1],
                    R[:, kx * H + ky: kx * H + ky + 125: 2],  # 63 rows
                    axis=AX.X, op=ALU.add)

            # cond chain (tiny matmuls, plain fp32)
            pc1 = gpsum.tile([128, 512], f32, tag="gp", name="gp")
            for i, k in enumerate(range(9)):
                nc.tensor.matmul(
                    pc1[0:32, 0:1], w1taps[:, 32 * k: 32 * k + 32],
                    Tt[:, k:k + 1], start=(i == 0), stop=(i == 8))
            cond1 = ppool.tile([32, 1], f32, tag="cond1", name="cond1")
            nc.scalar.activation(cond1[:], pc1[0:32, 0:1], ACT.Relu, bias=b1x2[:])
            pc2 = gpsum.tile([128, 512], f32, tag="gp", name="gp")
            nc.tensor.matmul(pc2[0:32, 0:1], cw2[:], cond1[:])
            cond2 = ppool.tile([32, 1], f32, tag="cond2", name="cond2")
            nc.scalar.activation(cond2[:], pc2[0:32, 0:1], ACT.Relu, bias=b2x2[:])

            # gen stationary [33, 2]: col s = cond2_s (rows 16s..16s+16), row
            # 32 = 1 (preset).  sgen[i, s] = maskc[i, s] * cond2[i]
            sgen = sgens[p]
            nc.vector.tensor_scalar(
                out=sgen[0:32, :], in0=maskc[:], scalar1=cond2[:, 0:1],
                scalar2=None, op0=ALU.mult)

            # generator matmuls -> gen_sb [2, 4736] (pw | dw | dbias), relu'd
            gen_sb = ppool.tile([2, GEN_W], bf16, tag="gen_sb", name="gen_sb")
            for i in range(10):
                c0 = 512 * i
                n = min(512, GEN_W - c0)
                gp = gpsum.tile([128, 512], f32, tag="gp", name="gp")
                nc.tensor.matmul(
                    gp[0:2, 0:n], sgen[:], rhs_gen[:, c0:c0 + n])
                if c0 >= 4608:  # last chunk: dw tail (relu) + dbias (no relu)
                    nc.scalar.activation(gen_sb[0:2, 4608:4672], gp[0:2, 0:64], ACT.Relu)
                    nc.scalar.copy(gen_sb[0:2, 4672:4736], gp[0:2, 64:128])
                elif i % 2 == 0:
                    nc.scalar.activation(gen_sb[0:2, c0:c0 + n], gp[0:2, 0:n], ACT.Relu)
                else:
                    nc.vector.tensor_scalar(
                        out=gen_sb[0:2, c0:c0 + n], in0=gp[0:2, 0:n],
                        scalar1=0.0, scalar2=None, op0=ALU.max)

            # rearrange to channel-major layouts
            pwcb = ppool.tile([128, O], bf16, tag="pwcb", name="pwcb")
            dwcb_h = ppool.tile([128, 9], bf16, tag="dwcb_h", name="dwcb_h")
            for s in range(2):
                nc.sync.dma_start(
                    pwcb[s * 64:(s + 1) * 64, :],
                    gen_sb[s:s + 1, 0:O * C].rearrange(
                        "s (c o) -> s c o", o=O, c=C))
                nc.sync.dma_start(
                    dwcb_h[s * 64:(s + 1) * 64, :],
                    gen_sb[s:s + 1, O * C:O * C + C * 9].rearrange(
                        "s (c k) -> s c k", c=C, k=9))
                nc.sync.dma_start(dbias2[s * 64:(s + 1) * 64, p:p + 1],
                                  gen_sb[s:s + 1, 4672:4736])
            # f32 copy (scalar operands of tensor_scalar must be f32)
            dwcb = ppool.tile([128, 9], f32, tag="dwcb", name="dwcb")
            nc.vector.tensor_copy(dwcb[:], dwcb_h[:])

            # S_k stationary: S[s*64+c, 128k + s*64+o] = pw[o,c]*dw[c,k]
            # (k-outer order so conv tap k can start as soon as block k lands)
            st = sts[p]
            for k in range(9):
                for s in range(2):
                    nc.vector.tensor_scalar(
                        out=st[s * 64:(s + 1) * 64,
                               128 * k + s * 64: 128 * k + s * 64 + 64],
                        in0=pwcb[s * 64:(s + 1) * 64, :],
                        scalar1=dwcb[s * 64:(s + 1) * 64, k:k + 1],
                        scalar2=None, op0=ALU.mult)

        def conv_group(p, g):
            """4 conv tiles (16 output rows) for pair p, group g."""
            xv = xview(p)
            st = sts[p]
            ob = obs[p]
            tiles = range(4 * g, 4 * g + 4)
            psums = {}
            for t in tiles:
                psums[t] = cpsum.tile([128, 512], f32, tag="cp", name="cp")
            for i in range(9):
                ky, kx = i // 3, i % 3
                lhsT = st[:, 128 * i: 128 * i + 128]
                for t in tiles:
                    h0 = 4 * t
                    # out rows h0..h0+3, tap (ky,kx): x rows h0+ky-1.., cols +kx-1
                    nc.tensor.matmul(
                        psums[t][:],
                        lhsT,
                        xv[:, h0 + ky: h0 + ky + 4, kx:kx + 128],
                        start=(i == 0), stop=(i == 8))
            for t in tiles:
                col = NT * p + t
                # copy pre-BN conv tile into SBUF (bf16) + per-partition sum
                nc.vector.tensor_scalar(
                    out=ob[:, 512 * t: 512 * t + 512],
                    in0=psums[t][:], scalar1=0.0, scalar2=0.0, op0=ALU.add,
                    op1=ALU.add, accum_out=sums[:, col:col + 1])
                sq = sqpool.tile([128, 512], bf16, tag="sq", name="sq")
                nc.scalar.activation(
                    sq[:], psums[t][:], ACT.Square,
                    accum_out=sumsqs[:, col:col + 1])

        # per-pair stats fixup, overlapped with the other pair's conv:
        # produces [128, 2] (sum, sumsq) with the dbias fold applied
        sum2 = stpool.tile([128, NPAIR], f32, tag="sum2", name="sum2")
        ssq2 = stpool.tile([128, NPAIR], f32, tag="ssq2", name="ssq2")
        dbias2f = stpool.tile([128, NPAIR], f32, tag="dbias2f", name="dbias2f")

        def pair_stats(p):
            pc = p * NT
            nc.vector.tensor_copy(dbias2f[:, p:p + 1], dbias2[:, p:p + 1])
            nc.vector.tensor_reduce(
                sum2[:, p:p + 1], sums[:, pc:pc + NT], axis=AX.X, op=ALU.add)
            nc.vector.tensor_reduce(
                ssq2[:, p:p + 1], sumsqs[:, pc:pc + NT], axis=AX.X, op=ALU.add)
            d16k = stpool.tile([128, 1], f32, tag="d16k", name="d16k", bufs=2)
            nc.vector.tensor_scalar(out=d16k[:], in0=dbias2f[:, p:p + 1],
                                    scalar1=float(HW), scalar2=None, op0=ALU.mult)
            t1 = stpool.tile([128, 1], f32, tag="t1", name="t1", bufs=2)
            # t1 = 2*d*sum + n*d^2 = d*(2*sum + n*d)
            nc.vector.tensor_scalar(out=t1[:], in0=sum2[:, p:p + 1], scalar1=2.0,
                                    scalar2=None, op0=ALU.mult)
            nc.vector.tensor_tensor(out=t1[:], in0=t1[:], in1=d16k[:], op=ALU.add)
            nc.vector.tensor_tensor(out=t1[:], in0=t1[:], in1=dbias2f[:, p:p + 1],
                                    op=ALU.mult)
            nc.vector.tensor_tensor(out=ssq2[:, p:p + 1], in0=ssq2[:, p:p + 1],
                                    in1=t1[:], op=ALU.add)
            nc.vector.tensor_tensor(out=sum2[:, p:p + 1], in0=sum2[:, p:p + 1],
                                    in1=d16k[:], op=ALU.add)

        # ---------------- main schedule ----------------
        # pair 0: load + pipelined row sums
        E0 = ppool.tile([128, H], f32, tag="E", name="E0")
        Od0 = ppool.tile([128, H], f32, tag="Od", name="Od0")
        for ch in range(4):
            load_seg(0, ch)
            eod_chunk(0, E0, Od0, ch)
        prep_pair(0, E0, Od0)

        E1 = ppool.tile([128, H], f32, tag="E", name="E1")
        Od1 = ppool.tile([128, H], f32, tag="Od", name="Od1")

        # pair 0 convs, with pair-1 load/prep interleaved so its DVE/DMA work
        # slots in behind pair-0 evictions without blocking them
        for g in range(8):
            conv_group(0, g)
            if g < 4:
                load_seg(1, g)
                eod_chunk(1, E1, Od1, g)
            elif g == 4:
                prep_pair(1, E1, Od1)
        pair_stats(0)
        for g in range(8):
            conv_group(1, g)
        pair_stats(1)

        # ---------------- BN statistics ----------------
        st128 = stpool.tile([128, 2], f32, tag="st128", name="st128")
        nc.vector.tensor_reduce(st128[:, 0:1], sum2[:], axis=AX.X, op=ALU.add)
        nc.vector.tensor_reduce(st128[:, 1:2], ssq2[:], axis=AX.X, op=ALU.add)
        # fold the two sample-halves: [128, 2] -> [64, 2]
        stlo = stpool.tile([64, 2], f32, tag="stlo", name="stlo")
        nc.sync.dma_start(stlo[:], st128[64:128, :])
        st64 = stpool.tile([64, 2], f32, tag="st64", name="st64")
        nc.vector.tensor_tensor(out=st64[:], in0=st128[0:64, :], in1=stlo[:],
                                op=ALU.add)

        # all-reduce across the 8 cores via DRAM bounce buffers
        cc_in = dpool.tile([64, 2], f32, tag="cc_in", name="cc_in")
        cc_out = dpool.tile([64, 2], f32, tag="cc_out", name="cc_out",
                            addr_space="Shared")
        nc.sync.dma_start(cc_in[:], st64[:])
        nc.gpsimd.collective_compute(
            "AllReduce", ALU.add,
            replica_groups=[list(range(NCORES))],
            ins=[cc_in[:].opt()], outs=[cc_out[:].opt()])
        # land the reduced stats on both sample-halves directly
        stg = stpool.tile([128, 2], f32, tag="stg", name="stg")
        nc.sync.dma_start(stg[0:64, :], cc_out[:])
        nc.sync.dma_start(stg[64:128, :], cc_out[:])

        # scale/shift: S = gamma/sqrt(var+eps) ; T2[:,p] = dbias*S + (beta - mean*S)
        gb2 = stpool.tile([128, 1], f32, tag="gb2", name="gb2")
        bb2 = stpool.tile([128, 1], f32, tag="bb2", name="bb2")
        nc.sync.dma_start(gb2[0:64, :], ins["gammab"])
        nc.sync.dma_start(gb2[64:128, :], ins["gammab"])
        nc.sync.dma_start(bb2[0:64, :], ins["betab"])
        nc.sync.dma_start(bb2[64:128, :], ins["betab"])

        ntot = float(BPC * NCORES * HW)
        ms = stpool.tile([128, 2], f32, tag="ms", name="ms")
        nc.vector.tensor_scalar(out=ms[:], in0=stg[:], scalar1=1.0 / ntot,
                                scalar2=None, op0=ALU.mult)  # (mean, E[x^2])
        var = stpool.tile([128, 1], f32, tag="var", name="var")
        nc.vector.tensor_tensor(out=var[:], in0=ms[:, 0:1], in1=ms[:, 0:1],
                                op=ALU.mult)
        nc.vector.tensor_tensor(out=var[:], in0=ms[:, 1:2], in1=var[:],
                                op=ALU.subtract)
        nc.vector.tensor_scalar(out=var[:], in0=var[:], scalar1=EPS,
                                scalar2=None, op0=ALU.add)
        std = stpool.tile([128, 1], f32, tag="std", name="std")
        nc.scalar.activation(std[:], var[:], ACT.Sqrt)
        inv = stpool.tile([128, 1], f32, tag="inv", name="inv")
        nc.vector.reciprocal(inv[:], std[:])
        Sb = stpool.tile([128, 1], f32, tag="Sb", name="Sb")
        nc.vector.tensor_tensor(out=Sb[:], in0=inv[:], in1=gb2[:], op=ALU.mult)
        Tb = stpool.tile([128, 1], f32, tag="Tb", name="Tb")
        nc.vector.tensor_tensor(out=Tb[:], in0=ms[:, 0:1], in1=Sb[:], op=ALU.mult)
        nc.vector.tensor_tensor(out=Tb[:], in0=bb2[:], in1=Tb[:], op=ALU.subtract)
        T2 = stpool.tile([128, NPAIR], f32, tag="T2", name="T2")
        nc.vector.tensor_scalar(out=T2[:], in0=dbias2f[:], scalar1=Sb[:],
                                scalar2=Tb[:], op0=ALU.mult, op1=ALU.add)

        # ---------------- final affine + store ----------------
        # alternate chunks between Vector and Scalar so the two engines share
        # the affine while the store DMA is the bottleneck
        CH = 2048
        for p in range(NPAIR):
            ob = obs[p]
            for i in range(HW // CH):
                c0 = CH * i
                if i % 2 == 0:
                    nc.vector.tensor_scalar(
                        out=ob[:, c0:c0 + CH], in0=ob[:, c0:c0 + CH],
                        scalar1=Sb[:], scalar2=T2[:, p:p + 1],
                        op0=ALU.mult, op1=ALU.add)
                else:
                    nc.scalar.activation(
                        ob[:, c0:c0 + CH], ob[:, c0:c0 + CH], ACT.Identity,
                        bias=T2[:, p:p + 1], scale=Sb[:])
                nc.sync.dma_start(
                    y_d[2 * p:2 * p + 2, :, c0:c0 + CH], ob[:, c0:c0 + CH])


# ---------------------------------------------------------------------------
# build + run
# ---------------------------------------------------------------------------
_CACHE = {}


def _build():
    if "nc" in _CACHE:
        return _CACHE["nc"]
    from concourse import bacc, mybir, tile

    nc = bacc.Bacc("TRN2", target_bir_lowering=False, debug=False,
                   num_devices=NCORES)
    f32 = mybir.dt.float32
    bf16 = mybir.dt.bfloat16
    ins = {
        "x": nc.dram_tensor("x", [BPC, C, H + 2, W + 2], bf16, kind="ExternalInput").ap(),
        "w1taps": nc.dram_tensor("w1taps", [128, 9 * 32], f32, kind="ExternalInput").ap(),
        "b1x2": nc.dram_tensor("b1x2", [32, 1], f32, kind="ExternalInput").ap(),
        "cw2": nc.dram_tensor("cw2", [32, 32], f32, kind="ExternalInput").ap(),
        "b2x2": nc.dram_tensor("b2x2", [32, 1], f32, kind="ExternalInput").ap(),
        "rhs_gen": nc.dram_tensor("rhs_gen", [33, GEN_W], bf16, kind="ExternalInput").ap(),
        "gammab": nc.dram_tensor("gammab", [64, 1], f32, kind="ExternalInput").ap(),
        "betab": nc.dram_tensor("betab", [64, 1], f32, kind="ExternalInput").ap(),
        "maskc": nc.dram_tensor("maskc", [32, 2], f32, kind="ExternalInput").ap(),
    }
    outs = {"y": nc.dram_tensor("y", [BPC, C, H, W], bf16, kind="ExternalOutput").ap()}
    with tile.TileContext(nc) as tc:
        body(tc, outs, ins)
    nc.compile()
    _CACHE["nc"] = nc
    return nc


def make_in_maps(inputs):
    x = np.asarray(inputs["x"], np.float32)
    xp = np.zeros((B, C, H + 2, W + 2), BF16)
    xp[:, :, 1:H + 1, 1:W + 1] = x.astype(BF16)
    consts = _prep_consts(inputs)
    in_maps = []
    for c in range(NCORES):
        m = {"x": np.ascontiguousarray(xp[BPC * c: BPC * (c + 1)])}
        m.update(consts)
        in_maps.append(m)
    return in_maps


def run(inputs, trace=False):
    from concourse.bass_utils import run_bass_kernel_spmd

    nc = _build()
    in_maps = make_in_maps(inputs)
    res = run_bass_kernel_spmd(nc, in_maps, core_ids=list(range(NCORES)),
                               trace=trace)
    y = np.concatenate(
        [np.asarray(res.results[c]["y"]).astype(np.float32)
         for c in range(NCORES)], axis=0)
    return y, res


def kernel(**inputs) -> np.ndarray:
    y, _ = run(inputs, trace=False)
    return y


# revision 14
# speedup vs baseline: 1.2616x; 1.0720x over previous
import sys

sys.path.insert(0, "/opt/trn_rl_repo")

import numpy as np
import ml_dtypes

BF16 = ml_dtypes.bfloat16

# ---- problem constants (hardcoded; kernel.py must be self-contained) ----
B, C, O, KK, H, W = 32, 64, 64, 3, 128, 128
COND = 16
NCORES = 8
BPC = B // NCORES          # samples per core = 4
NPAIR = BPC // 2           # sample-pairs per core = 2
HW = H * W                 # 16384
NT = H // 4                # 32 conv tiles (4 rows x 128 cols) per pair
WP = W + 2                 # 130: padded row width (zero col left/right)
XELEMS = WP * (H + 2)      # padded x elems per channel
GEN_W = O * C + C * KK * KK + O   # 4096 + 576 + 64 = 4736 generator outputs
NVALID = 63 * 63           # VALID conv output positions of the cond conv
EPS = 1e-5

# pad-row segments, aligned so E/Od chunk ch only needs segment ch
ROWSEG = [0, 17, 33, 49, 65, 81, 97, 113, H + 2]
NSEG = len(ROWSEG) - 1


# ---------------------------------------------------------------------------
# host-side constant prep (numpy only)
# ---------------------------------------------------------------------------
def _prep_consts(inp):
    f32 = np.float32
    cg_w1 = np.asarray(inp["cg_w1"], f32)      # [16, 64, 3, 3]
    cg_b1 = np.asarray(inp["cg_b1"], f32)      # [16]
    cg_w2 = np.asarray(inp["cg_w2"], f32)      # [16, 16]
    cg_b2 = np.asarray(inp["cg_b2"], f32)      # [16]
    wg_w = np.asarray(inp["wg_w"], f32)        # [576, 16]
    wg_b = np.asarray(inp["wg_b"], f32)        # [576]
    pg_w = np.asarray(inp["pg_w"], f32)        # [4096, 16]
    pg_b = np.asarray(inp["pg_b"], f32)        # [4096]
    bg_w = np.asarray(inp["bg_w"], f32)        # [64, 16]
    bg_b = np.asarray(inp["bg_b"], f32)        # [64]

    # cond-conv taps folded with the 1/3969 spatial mean:
    # w1taps[s*64+ci, 32*k + s*16+co] = cg_w1[co, ci, ky, kx] / 3969
    w1taps = np.zeros((128, 9 * 32), f32)
    for k in range(9):
        ky, kx = k // 3, k % 3
        blk = (cg_w1[:, :, ky, kx] / NVALID).T  # [ci, co]
        for s in range(2):
            w1taps[s * 64:(s + 1) * 64, 32 * k + s * 16: 32 * k + s * 16 + 16] = blk

    b1x2 = np.concatenate([cg_b1, cg_b1]).reshape(32, 1)
    b2x2 = np.concatenate([cg_b2, cg_b2]).reshape(32, 1)

    cw2 = np.zeros((32, 32), f32)
    for s in range(2):
        cw2[s * 16:(s + 1) * 16, s * 16:(s + 1) * 16] = cg_w2.T  # [ci, co]

    # generator moving operand: rows 0-15 and 16-31 both hold G^T, row 32 bias.
    # pw block stored c-major (flat index c*64+o) so the later SBUF rearrange
    # DMA has a contiguous inner dim.
    pg_w_co = pg_w.reshape(O, C, COND).transpose(1, 0, 2).reshape(O * C, COND)
    pg_b_co = pg_b.reshape(O, C).T.reshape(-1)
    G = np.concatenate([pg_w_co, wg_w, bg_w], axis=0)            # [4736, 16]
    gbias = np.concatenate([pg_b_co, wg_b, bg_b])                # [4736]
    rhs_gen = np.zeros((33, GEN_W), f32)
    rhs_gen[0:16] = G.T
    rhs_gen[16:32] = G.T
    rhs_gen[32] = gbias

    gammab = np.asarray(inp["bn_gamma"], f32).reshape(64, 1)
    betab = np.asarray(inp["bn_beta"], f32).reshape(64, 1)

    # maskc[i, s] = 1 if i // 16 == s  (for building the gen stationary)
    maskc = np.zeros((32, 2), f32)
    maskc[0:16, 0] = 1.0
    maskc[16:32, 1] = 1.0

    return {
        "w1taps": w1taps, "b1x2": b1x2, "cw2": cw2, "b2x2": b2x2,
        "rhs_gen": rhs_gen.astype(BF16), "gammab": gammab, "betab": betab,
        "maskc": maskc,
    }


# ---------------------------------------------------------------------------
# kernel body: emits one core's program under TileContext
# ---------------------------------------------------------------------------
def body(tc, outs, ins):
    import concourse.bass as bass
    from concourse import mybir

    nc = tc.nc
    f32 = mybir.dt.float32
    bf16 = mybir.dt.bfloat16
    AX = mybir.AxisListType
    ALU = mybir.AluOpType
    ACT = mybir.ActivationFunctionType

    x_d = ins["x"].rearrange("b c h w -> b c (h w)")      # [4, 64, 16900] bf16
    y_d = outs["y"].rearrange("b c h w -> b c (h w)")     # [4, 64, 16384] bf16

    with (
        tc.tile_pool(name="xbuf", bufs=1) as xpool,
        tc.tile_pool(name="obuf", bufs=1) as opool,
        tc.tile_pool(name="consts", bufs=1) as cpool,
        tc.tile_pool(name="pairbuf", bufs=2) as ppool,
        tc.tile_pool(name="sq", bufs=2) as sqpool,
        tc.tile_pool(name="stats", bufs=1) as stpool,
        tc.tile_pool(name="cpsum", bufs=6, space="PSUM") as cpsum,
        tc.tile_pool(name="gpsum", bufs=2, space="PSUM") as gpsum,
        tc.tile_pool(name="dram", bufs=1, space="DRAM") as dpool,
    ):
        # ---- constants into SBUF ----
        w1taps = cpool.tile([128, 9 * 32], f32, tag="w1taps", name="w1taps")
        b1x2 = cpool.tile([32, 1], f32, tag="b1x2", name="b1x2")
        cw2 = cpool.tile([32, 32], f32, tag="cw2", name="cw2")
        b2x2 = cpool.tile([32, 1], f32, tag="b2x2", name="b2x2")
        rhs_gen = cpool.tile([33, GEN_W], bf16, tag="rhs_gen", name="rhs_gen")
        gammab = cpool.tile([64, 1], f32, tag="gammab", name="gammab")
        betab = cpool.tile([64, 1], f32, tag="betab", name="betab")
        maskc = cpool.tile([32, 2], f32, tag="maskc", name="maskc")
        for t_, n_ in ((w1taps, "w1taps"), (b1x2, "b1x2"), (cw2, "cw2"),
                       (b2x2, "b2x2"), (gammab, "gammab"), (betab, "betab"),
                       (rhs_gen, "rhs_gen"), (maskc, "maskc")):
            nc.sync.dma_start(t_[:], ins[n_])

        # ---- persistent state ----
        xbs = [xpool.tile([128, XELEMS], bf16, tag=f"xb{p}", name=f"xb{p}")
               for p in range(NPAIR)]
        obs = [opool.tile([128, HW], bf16, tag=f"ob{p}", name=f"ob{p}")
               for p in range(NPAIR)]
        sts = [cpool.tile([128, 9 * 128], bf16, tag=f"st{p}", name=f"st{p}")
               for p in range(NPAIR)]
        sgens = [cpool.tile([33, 2], bf16, tag=f"sgen{p}", name=f"sgen{p}")
                 for p in range(NPAIR)]
        # zero the stationaries once (block structure is identical per pair);
        # set the all-ones bias row of the gen stationaries once
        for p in range(NPAIR):
            nc.gpsimd.memset(sts[p][:], 0.0)
            nc.gpsimd.memset(sgens[p][0:32, :], 0.0)
            nc.gpsimd.memset(sgens[p][32:33, :], 1.0)

        dbias2 = stpool.tile([128, NPAIR], bf16, tag="dbias2", name="dbias2")
        sums = stpool.tile([128, NPAIR * NT], f32, tag="sums", name="sums")
        sumsqs = stpool.tile([128, NPAIR * NT], f32, tag="sumsqs", name="sumsqs")

        def xview(p):
            # padded x: row h of the image lives at xv[:, h+1, 1:129]
            return xbs[p][:].rearrange("p (r w) -> p r w", r=H + 2, w=WP)

        def load_seg(p, ch):
            e0, e1 = WP * ROWSEG[ch], WP * ROWSEG[ch + 1]
            nc.sync.dma_start(xbs[p][:, e0:e1], x_d[2 * p:2 * p + 2, :, e0:e1])

        # E/Od row sums (even / odd columns); chunk ch covers image rows
        # 16ch..16ch+15 and only needs x segment ch.  eng selects the engine
        # (vector for the critical pair-0 prep, gpsimd to keep DVE free while
        # convs run).
        def eod_chunk(p, E, Od, ch, eng):
            xv = xview(p)
            r0 = 1 + 16 * ch
            eng.tensor_reduce(
                E[:, 16 * ch: 16 * ch + 16],
                xv[:, r0:r0 + 16, 1:128:2], axis=AX.X, op=ALU.add)
            eng.tensor_reduce(
                Od[:, 16 * ch: 16 * ch + 16],
                xv[:, r0:r0 + 16, 2:129:2], axis=AX.X, op=ALU.add)

        def prep_pair(p, E, Od):
            """cond generator -> dynamic weights -> S_k stationary for pair p.

            E/Od row sums must already be computed (pipelined with the load).
            """
            xv = xview(p)
            # border columns 1, 127, 128 as f32 for the strided col sums
            bord = ppool.tile([128, 3 * H], f32, tag="bord", name="bord")
            colv = xv[:, 1:1 + H, :]
            nc.vector.tensor_copy(bord[:].rearrange("p (c r) -> p c r", c=3, r=H)[:, 0, :],
                                  colv[:, :, 127])
            nc.vector.tensor_copy(bord[:].rearrange("p (c r) -> p c r", c=3, r=H)[:, 1, :],
                                  colv[:, :, 128])
            nc.vector.tensor_copy(bord[:].rearrange("p (c r) -> p c r", c=3, r=H)[:, 2, :],
                                  colv[:, :, 1])
            # per-row strided col sums for kx = 0,1,2
            R = ppool.tile([128, 3 * H], f32, tag="R", name="R")
            nc.vector.tensor_tensor(
                out=R[:, 0:H], in0=E[:], in1=bord[:, 0:H], op=ALU.subtract)
            nc.vector.tensor_tensor(
                out=R[:, H:2 * H], in0=Od[:], in1=bord[:, H:2 * H], op=ALU.subtract)
            nc.vector.tensor_tensor(
                out=R[:, 2 * H:3 * H], in0=E[:], in1=bord[:, 2 * H:3 * H],
                op=ALU.subtract)
            # T[ci, k] strided row sums
            Tt = ppool.tile([128, 9], f32, tag="Tt", name="Tt")
            for k in range(9):
                ky, kx = k // 3, k % 3
                nc.vector.tensor_reduce(
                    Tt[:, k:k + 1],
                    R[:, kx * H + ky: kx * H + ky + 125: 2],  # 63 rows
                    axis=AX.X, op=ALU.add)

            # cond chain (tiny matmuls, plain fp32)
            pc1 = gpsum.tile([128, 512], f32, tag="gp", name="gp")
            for i, k in enumerate(range(9)):
                nc.tensor.matmul(
                    pc1[0:32, 0:1], w1taps[:, 32 * k: 32 * k + 32],
                    Tt[:, k:k + 1], start=(i == 0), stop=(i == 8))
            cond1 = ppool.tile([32, 1], f32, tag="cond1", name="cond1")
            nc.scalar.activation(cond1[:], pc1[0:32, 0:1], ACT.Relu, bias=b1x2[:])
            pc2 = gpsum.tile([128, 512], f32, tag="gp", name="gp")
            nc.tensor.matmul(pc2[0:32, 0:1], cw2[:], cond1[:])
            cond2 = ppool.tile([32, 1], f32, tag="cond2", name="cond2")
            nc.scalar.activation(cond2[:], pc2[0:32, 0:1], ACT.Relu, bias=b2x2[:])

            # gen stationary [33, 2]: col s = cond2_s (rows 16s..16s+16), row
            # 32 = 1 (preset).  sgen[i, s] = maskc[i, s] * cond2[i]
            sgen = sgens[p]
            nc.vector.tensor_scalar(
                out=sgen[0:32, :], in0=maskc[:], scalar1=cond2[:, 0:1],
                scalar2=None, op0=ALU.mult)

            # generator matmuls -> gen_sb [2, 4736] (pw | dw | dbias), relu'd
            gen_sb = ppool.tile([2, GEN_W], bf16, tag="gen_sb", name="gen_sb")
            for i in range(10):
                c0 = 512 * i
                n = min(512, GEN_W - c0)
                gp = gpsum.tile([128, 512], f32, tag="gp", name="gp")
                nc.tensor.matmul(
                    gp[0:2, 0:n], sgen[:], rhs_gen[:, c0:c0 + n])
                if c0 >= 4608:  # last chunk: dw tail (relu) + dbias (no relu)
                    nc.scalar.activation(gen_sb[0:2, 4608:4672], gp[0:2, 0:64], ACT.Relu)
                    nc.scalar.copy(gen_sb[0:2, 4672:4736], gp[0:2, 64:128])
                elif i % 2 == 0:
                    nc.scalar.activation(gen_sb[0:2, c0:c0 + n], gp[0:2, 0:n], ACT.Relu)
                else:
                    nc.vector.tensor_scalar(
                        out=gen_sb[0:2, c0:c0 + n], in0=gp[0:2, 0:n],
                        scalar1=0.0, scalar2=None, op0=ALU.max)

            # rearrange to channel-major layouts
            pwcb = ppool.tile([128, O], bf16, tag="pwcb", name="pwcb")
            dwcb_h = ppool.tile([128, 9], bf16, tag="dwcb_h", name="dwcb_h")
            for s in range(2):
                nc.sync.dma_start(
                    pwcb[s * 64:(s + 1) * 64, :],
                    gen_sb[s:s + 1, 0:O * C].rearrange(
                        "s (c o) -> s c o", o=O, c=C))
                nc.sync.dma_start(
                    dwcb_h[s * 64:(s + 1) * 64, :],
                    gen_sb[s:s + 1, O * C:O * C + C * 9].rearrange(
                        "s (c k) -> s c k", c=C, k=9))
                nc.sync.dma_start(dbias2[s * 64:(s + 1) * 64, p:p + 1],
                                  gen_sb[s:s + 1, 4672:4736])
            # f32 copy (scalar operands of tensor_scalar must be f32)
            dwcb = ppool.tile([128, 9], f32, tag="dwcb", name="dwcb")
            nc.vector.tensor_copy(dwcb[:], dwcb_h[:])

            # S_k stationary: S[s*64+c, 128k + s*64+o] = pw[o,c]*dw[c,k]
            # (k-outer order so conv tap k can start as soon as block k lands)
            st = sts[p]
            for k in range(9):
                for s in range(2):
                    nc.vector.tensor_scalar(
                        out=st[s * 64:(s + 1) * 64,
                               128 * k + s * 64: 128 * k + s * 64 + 64],
                        in0=pwcb[s * 64:(s + 1) * 64, :],
                        scalar1=dwcb[s * 64:(s + 1) * 64, k:k + 1],
                        scalar2=None, op0=ALU.mult)

        def conv_group(p, tiles):
            """conv tiles (4 output rows each) for pair p."""
            xv = xview(p)
            st = sts[p]
            ob = obs[p]
            psums = {}
            for t in tiles:
                psums[t] = cpsum.tile([128, 512], f32, tag="cp", name="cp")
            for i in range(9):
                ky, kx = i // 3, i % 3
                lhsT = st[:, 128 * i: 128 * i + 128]
                for t in tiles:
                    h0 = 4 * t
                    # out rows h0..h0+3, tap (ky,kx): x rows h0+ky-1.., cols +kx-1
                    nc.tensor.matmul(
                        psums[t][:],
                        lhsT,
                        xv[:, h0 + ky: h0 + ky + 4, kx:kx + 128],
                        start=(i == 0), stop=(i == 8))
            for t in tiles:
                col = NT * p + t
                # copy pre-BN conv tile into SBUF (bf16) + per-partition sum
                nc.vector.tensor_scalar(
                    out=ob[:, 512 * t: 512 * t + 512],
                    in0=psums[t][:], scalar1=0.0, scalar2=0.0, op0=ALU.add,
                    op1=ALU.add, accum_out=sums[:, col:col + 1])
                sq = sqpool.tile([128, 512], bf16, tag="sq", name="sq")
                nc.scalar.activation(
                    sq[:], psums[t][:], ACT.Square,
                    accum_out=sumsqs[:, col:col + 1])

        # per-pair stats fixup, overlapped with the other pair's conv:
        # produces [128, 2] (sum, sumsq) with the dbias fold applied
        sum2 = stpool.tile([128, NPAIR], f32, tag="sum2", name="sum2")
        ssq2 = stpool.tile([128, NPAIR], f32, tag="ssq2", name="ssq2")
        dbias2f = stpool.tile([128, NPAIR], f32, tag="dbias2f", name="dbias2f")

        def pair_stats(p):
            pc = p * NT
            nc.vector.tensor_copy(dbias2f[:, p:p + 1], dbias2[:, p:p + 1])
            nc.vector.tensor_reduce(
                sum2[:, p:p + 1], sums[:, pc:pc + NT], axis=AX.X, op=ALU.add)
            nc.vector.tensor_reduce(
                ssq2[:, p:p + 1], sumsqs[:, pc:pc + NT], axis=AX.X, op=ALU.add)
            d16k = stpool.tile([128, 1], f32, tag="d16k", name="d16k", bufs=2)
            nc.vector.tensor_scalar(out=d16k[:], in0=dbias2f[:, p:p + 1],
                                    scalar1=float(HW), scalar2=None, op0=ALU.mult)
            t1 = stpool.tile([128, 1], f32, tag="t1", name="t1", bufs=2)
            # t1 = 2*d*sum + n*d^2 = d*(2*sum + n*d)
            nc.vector.tensor_scalar(out=t1[:], in0=sum2[:, p:p + 1], scalar1=2.0,
                                    scalar2=None, op0=ALU.mult)
            nc.vector.tensor_tensor(out=t1[:], in0=t1[:], in1=d16k[:], op=ALU.add)
            nc.vector.tensor_tensor(out=t1[:], in0=t1[:], in1=dbias2f[:, p:p + 1],
                                    op=ALU.mult)
            nc.vector.tensor_tensor(out=ssq2[:, p:p + 1], in0=ssq2[:, p:p + 1],
                                    in1=t1[:], op=ALU.add)
            nc.vector.tensor_tensor(out=sum2[:, p:p + 1], in0=sum2[:, p:p + 1],
                                    in1=d16k[:], op=ALU.add)

        # ---------------- warmups ----------------
        # dummy collective to warm the CC stream / gpsimd path while the x
        # loads run, and activation-table preloads (Sqrt/Identity are needed
        # on the post-collective critical path)
        wsb = stpool.tile([64, 2], f32, tag="wsb", name="wsb")
        nc.gpsimd.memset(wsb[:], 0.0)
        wu_in = dpool.tile([64, 2], f32, tag="wu_in", name="wu_in")
        wu_out = dpool.tile([64, 2], f32, tag="wu_out", name="wu_out")
        nc.sync.dma_start(wu_in[:], wsb[:])
        nc.gpsimd.collective_compute(
            "AllReduce", ALU.add,
            replica_groups=[list(range(NCORES))],
            ins=[wu_in[:].opt()], outs=[wu_out[:].opt()])
        dumt = stpool.tile([64, 2], f32, tag="dumt", name="dumt")
        nc.scalar.activation(dumt[:, 0:1], gammab[:], ACT.Sqrt)
        nc.scalar.activation(dumt[:, 1:2], gammab[:], ACT.Identity)

        # ---------------- main schedule ----------------
        # pair 0: load + pipelined row sums
        E0 = ppool.tile([128, H], f32, tag="E", name="E0")
        Od0 = ppool.tile([128, H], f32, tag="Od", name="Od0")
        for ch in range(NSEG):
            load_seg(0, ch)
            eod_chunk(0, E0, Od0, ch, nc.vector)
        prep_pair(0, E0, Od0)

        E1 = ppool.tile([128, H], f32, tag="E", name="E1")
        Od1 = ppool.tile([128, H], f32, tag="Od", name="Od1")

        # 3-tile conv groups double-buffer perfectly in the 6 PSUM banks
        groups = [list(range(3 * g, min(3 * g + 3, NT))) for g in range(11)]
        # pair 0 convs, with pair-1 load/prep interleaved; pair-1 row sums go
        # to gpsimd so they never block pair-0 PSUM evictions on DVE
        for g, tiles in enumerate(groups):
            conv_group(0, tiles)
            if g < NSEG:
                load_seg(1, g)
                eod_chunk(1, E1, Od1, g, nc.vector)
            elif g == NSEG:
                prep_pair(1, E1, Od1)
        pair_stats(0)
        for tiles in groups:
            conv_group(1, tiles)
        pair_stats(1)

        # ---------------- BN statistics ----------------
        st128 = stpool.tile([128, 2], f32, tag="st128", name="st128")
        nc.vector.tensor_reduce(st128[:, 0:1], sum2[:], axis=AX.X, op=ALU.add)
        nc.vector.tensor_reduce(st128[:, 1:2], ssq2[:], axis=AX.X, op=ALU.add)
        # fold the two sample-halves: [128, 2] -> [64, 2]
        stlo = stpool.tile([64, 2], f32, tag="stlo", name="stlo")
        nc.sync.dma_start(stlo[:], st128[64:128, :])
        st64 = stpool.tile([64, 2], f32, tag="st64", name="st64")
        nc.vector.tensor_tensor(out=st64[:], in0=st128[0:64, :], in1=stlo[:],
                                op=ALU.add)

        # all-reduce across the 8 cores via DRAM bounce buffers
        cc_in = dpool.tile([64, 2], f32, tag="cc_in", name="cc_in")
        cc_out = dpool.tile([64, 2], f32, tag="cc_out", name="cc_out")
        nc.sync.dma_start(cc_in[:], st64[:])
        nc.gpsimd.collective_compute(
            "AllReduce", ALU.add,
            replica_groups=[list(range(NCORES))],
            ins=[cc_in[:].opt()], outs=[cc_out[:].opt()])
        # land the reduced stats on both sample-halves directly
        stg = stpool.tile([128, 2], f32, tag="stg", name="stg")
        nc.sync.dma_start(stg[0:64, :], cc_out[:])
        nc.sync.dma_start(stg[64:128, :], cc_out[:])

        # scale/shift: S = gamma/sqrt(var+eps) ; T2[:,p] = dbias*S + (beta - mean*S)
        gb2 = stpool.tile([128, 1], f32, tag="gb2", name="gb2")
        bb2 = stpool.tile([128, 1], f32, tag="bb2", name="bb2")
        nc.sync.dma_start(gb2[0:64, :], ins["gammab"])
        nc.sync.dma_start(gb2[64:128, :], ins["gammab"])
        nc.sync.dma_start(bb2[0:64, :], ins["betab"])
        nc.sync.dma_start(bb2[64:128, :], ins["betab"])

        ntot = float(BPC * NCORES * HW)
        ms = stpool.tile([128, 2], f32, tag="ms", name="ms")
        nc.vector.tensor_scalar(out=ms[:], in0=stg[:], scalar1=1.0 / ntot,
                                scalar2=None, op0=ALU.mult)  # (mean, E[x^2])
        var = stpool.tile([128, 1], f32, tag="var", name="var")
        nc.vector.tensor_tensor(out=var[:], in0=ms[:, 0:1], in1=ms[:, 0:1],
                                op=ALU.mult)
        nc.vector.tensor_tensor(out=var[:], in0=ms[:, 1:2], in1=var[:],
                                op=ALU.subtract)
        nc.vector.tensor_scalar(out=var[:], in0=var[:], scalar1=EPS,
                                scalar2=None, op0=ALU.add)
        std = stpool.tile([128, 1], f32, tag="std", name="std")
        nc.scalar.activation(std[:], var[:], ACT.Sqrt)
        inv = stpool.tile([128, 1], f32, tag="inv", name="inv")
        nc.vector.reciprocal(inv[:], std[:])
        Sb = stpool.tile([128, 1], f32, tag="Sb", name="Sb")
        nc.vector.tensor_tensor(out=Sb[:], in0=inv[:], in1=gb2[:], op=ALU.mult)
        Tb = stpool.tile([128, 1], f32, tag="Tb", name="Tb")
        nc.vector.tensor_tensor(out=Tb[:], in0=ms[:, 0:1], in1=Sb[:], op=ALU.mult)
        nc.vector.tensor_tensor(out=Tb[:], in0=bb2[:], in1=Tb[:], op=ALU.subtract)
        T2 = stpool.tile([128, NPAIR], f32, tag="T2", name="T2")
        nc.vector.tensor_scalar(out=T2[:], in0=dbias2f[:], scalar1=Sb[:],
                                scalar2=Tb[:], op0=ALU.mult, op1=ALU.add)

        # ---------------- final affine + store ----------------
        # alternate chunks between Vector and Scalar so the two engines share
        # the affine while the store DMA is the bottleneck
        CH = 2048
        for i in range(HW // CH):
            c0 = CH * i
            for p in range(NPAIR):
                ob = obs[p]
                if (i + p) % 2 == 0:
                    nc.vector.tensor_scalar(
                        out=ob[:, c0:c0 + CH], in0=ob[:, c0:c0 + CH],
                        scalar1=Sb[:], scalar2=T2[:, p:p + 1],
                        op0=ALU.mult, op1=ALU.add)
                else:
                    nc.scalar.activation(
                        ob[:, c0:c0 + CH], ob[:, c0:c0 + CH], ACT.Identity,
                        bias=T2[:, p:p + 1], scale=Sb[:])
                nc.sync.dma_start(
                    y_d[2 * p:2 * p + 2, :, c0:c0 + CH], ob[:, c0:c0 + CH])


# ---------------------------------------------------------------------------
# build + run
# ---------------------------------------------------------------------------
_CACHE = {}


def _build():
    if "nc" in _CACHE:
        return _CACHE["nc"]
    from concourse import bacc, mybir, tile

    nc = bacc.Bacc("TRN2", target_bir_lowering=False, debug=False,
                   num_devices=NCORES)
    f32 = mybir.dt.float32
    bf16 = mybir.dt.bfloat16
    ins = {
        "x": nc.dram_tensor("x", [BPC, C, H + 2, W + 2], bf16, kind="ExternalInput").ap(),
        "w1taps": nc.dram_tensor("w1taps", [128, 9 * 32], f32, kind="ExternalInput").ap(),
        "b1x2": nc.dram_tensor("b1x2", [32, 1], f32, kind="ExternalInput").ap(),
        "cw2": nc.dram_tensor("cw2", [32, 32], f32, kind="ExternalInput").ap(),
        "b2x2": nc.dram_tensor("b2x2", [32, 1], f32, kind="ExternalInput").ap(),
        "rhs_gen": nc.dram_tensor("rhs_gen", [33, GEN_W], bf16, kind="ExternalInput").ap(),
        "gammab": nc.dram_tensor("gammab", [64, 1], f32, kind="ExternalInput").ap(),
        "betab": nc.dram_tensor("betab", [64, 1], f32, kind="ExternalInput").ap(),
        "maskc": nc.dram_tensor("maskc", [32, 2], f32, kind="ExternalInput").ap(),
    }
    outs = {"y": nc.dram_tensor("y", [BPC, C, H, W], bf16, kind="ExternalOutput").ap()}
    with tile.TileContext(nc) as tc:
        body(tc, outs, ins)
    nc.compile()
    _CACHE["nc"] = nc
    return nc


def make_in_maps(inputs):
    x = np.asarray(inputs["x"], np.float32)
    xp = np.zeros((B, C, H + 2, W + 2), BF16)
    xp[:, :, 1:H + 1, 1:W + 1] = x.astype(BF16)
    consts = _prep_consts(inputs)
    in_maps = []
    for c in range(NCORES):
        m = {"x": np.ascontiguousarray(xp[BPC * c: BPC * (c + 1)])}
        m.update(consts)
        in_maps.append(m)
    return in_maps


def run(inputs, trace=False):
    from concourse.bass_utils import run_bass_kernel_spmd

    nc = _build()
    in_maps = make_in_maps(inputs)
    res = run_bass_kernel_spmd(nc, in_maps, core_ids=list(range(NCORES)),
                               trace=trace)
    y = np.concatenate(
        [np.asarray(res.results[c]["y"]).astype(np.float32)
         for c in range(NCORES)], axis=0)
    return y, res


def kernel(**inputs) -> np.ndarray:
    y, _ = run(inputs, trace=False)
    return y


# revision 24
# speedup vs baseline: 1.3161x; 1.0432x over previous
import sys

sys.path.insert(0, "/opt/trn_rl_repo")

import numpy as np
import ml_dtypes

BF16 = ml_dtypes.bfloat16

# ---- problem constants (hardcoded; kernel.py must be self-contained) ----
B, C, O, KK, H, W = 32, 64, 64, 3, 128, 128
COND = 16
NCORES = 8
BPC = B // NCORES          # samples per core = 4
NPAIR = BPC // 2           # sample-pairs per core = 2
HW = H * W                 # 16384
NT = H // 4                # 32 conv tiles (4 rows x 128 cols) per pair
WP = W + 2                 # 130: padded row width (zero col left/right)
XELEMS = WP * (H + 2)      # padded x elems per channel
GEN_W = O * C + C * KK * KK + O   # 4096 + 576 + 64 = 4736 generator outputs
NVALID = 63 * 63           # VALID conv output positions of the cond conv
EPS = 1e-5

# pad-row segments for the x load; E/Od chunk ch (16 rows) needs seg ch//2
ROWSEG = [0, 33, 65, 97, H + 2]
NSEG = len(ROWSEG) - 1


# ---------------------------------------------------------------------------
# host-side constant prep (numpy only)
# ---------------------------------------------------------------------------
def _prep_consts(inp):
    f32 = np.float32
    cg_w1 = np.asarray(inp["cg_w1"], f32)      # [16, 64, 3, 3]
    cg_b1 = np.asarray(inp["cg_b1"], f32)      # [16]
    cg_w2 = np.asarray(inp["cg_w2"], f32)      # [16, 16]
    cg_b2 = np.asarray(inp["cg_b2"], f32)      # [16]
    wg_w = np.asarray(inp["wg_w"], f32)        # [576, 16]
    wg_b = np.asarray(inp["wg_b"], f32)        # [576]
    pg_w = np.asarray(inp["pg_w"], f32)        # [4096, 16]
    pg_b = np.asarray(inp["pg_b"], f32)        # [4096]
    bg_w = np.asarray(inp["bg_w"], f32)        # [64, 16]
    bg_b = np.asarray(inp["bg_b"], f32)        # [64]

    # cond-conv taps folded with the 1/3969 spatial mean:
    # w1taps[s*64+ci, 32*k + s*16+co] = cg_w1[co, ci, ky, kx] / 3969
    w1taps = np.zeros((128, 9 * 32), f32)
    for k in range(9):
        ky, kx = k // 3, k % 3
        blk = (cg_w1[:, :, ky, kx] / NVALID).T  # [ci, co]
        for s in range(2):
            w1taps[s * 64:(s + 1) * 64, 32 * k + s * 16: 32 * k + s * 16 + 16] = blk

    b1x2 = np.concatenate([cg_b1, cg_b1]).reshape(32, 1)
    b2x2 = np.concatenate([cg_b2, cg_b2]).reshape(32, 1)

    cw2 = np.zeros((32, 32), f32)
    for s in range(2):
        cw2[s * 16:(s + 1) * 16, s * 16:(s + 1) * 16] = cg_w2.T  # [ci, co]

    # generator moving operand: rows 0-15 and 16-31 both hold G^T, row 32 bias.
    # pw block stored c-major (flat index c*64+o) so the later SBUF rearrange
    # DMA has a contiguous inner dim.
    pg_w_co = pg_w.reshape(O, C, COND).transpose(1, 0, 2).reshape(O * C, COND)
    pg_b_co = pg_b.reshape(O, C).T.reshape(-1)
    G = np.concatenate([pg_w_co, wg_w, bg_w], axis=0)            # [4736, 16]
    gbias = np.concatenate([pg_b_co, wg_b, bg_b])                # [4736]
    rhs_gen = np.zeros((33, GEN_W), f32)
    rhs_gen[0:16] = G.T
    rhs_gen[16:32] = G.T
    rhs_gen[32] = gbias

    gammab = np.asarray(inp["bn_gamma"], f32).reshape(64, 1)
    betab = np.asarray(inp["bn_beta"], f32).reshape(64, 1)

    # maskc[i, s] = 1 if i // 16 == s  (for building the gen stationary)
    maskc = np.zeros((32, 2), f32)
    maskc[0:16, 0] = 1.0
    maskc[16:32, 1] = 1.0

    # partition-fold stationary: out[m] = in[m] + in[m+64]
    fold128 = np.zeros((128, 64), f32)
    for p in range(128):
        fold128[p, p % 64] = 1.0

    return {
        "w1taps": w1taps, "b1x2": b1x2, "cw2": cw2, "b2x2": b2x2,
        "rhs_gen": rhs_gen.astype(BF16), "gammab": gammab, "betab": betab,
        "maskc": maskc, "fold128": fold128,
    }


# ---------------------------------------------------------------------------
# kernel body: emits one core's program under TileContext
# ---------------------------------------------------------------------------
def body(tc, outs, ins):
    import concourse.bass as bass
    from concourse import mybir

    nc = tc.nc
    f32 = mybir.dt.float32
    bf16 = mybir.dt.bfloat16
    AX = mybir.AxisListType
    ALU = mybir.AluOpType
    ACT = mybir.ActivationFunctionType

    x_d = ins["x"].rearrange("b c h w -> b c (h w)")      # [4, 64, 16900] bf16
    y_d = outs["y"].rearrange("b c h w -> b c (h w)")     # [4, 64, 16384] bf16

    with (
        tc.tile_pool(name="xbuf", bufs=1) as xpool,
        tc.tile_pool(name="obuf", bufs=1) as opool,
        tc.tile_pool(name="consts", bufs=1) as cpool,
        tc.tile_pool(name="pairbuf", bufs=2) as ppool,
        tc.tile_pool(name="sq", bufs=2) as sqpool,
        tc.tile_pool(name="stats", bufs=1) as stpool,
        tc.tile_pool(name="cpsum", bufs=6, space="PSUM") as cpsum,
        tc.tile_pool(name="gpsum", bufs=2, space="PSUM") as gpsum,
        tc.tile_pool(name="dram", bufs=1, space="DRAM") as dpool,
    ):
        # ---- persistent state (x tiles declared first; their loads are the
        # critical path and go out on the sync queue before anything else) ----
        xbs = [xpool.tile([128, XELEMS], bf16, tag=f"xb{p}", name=f"xb{p}")
               for p in range(NPAIR)]

        def load_seg0(p, ch):
            e0, e1 = WP * ROWSEG[ch], WP * ROWSEG[ch + 1]
            nc.sync.dma_start(xbs[p][:, e0:e1],
                              ins["x"].rearrange("b c h w -> b c (h w)")
                              [2 * p:2 * p + 2, :, e0:e1])

        for ch in range(NSEG):
            load_seg0(0, ch)

        # ---- constants into SBUF (issued from the scalar engine's DMA queue
        # so they don't delay the x loads) ----
        w1taps = cpool.tile([128, 9 * 32], f32, tag="w1taps", name="w1taps")
        b1x2 = cpool.tile([32, 1], f32, tag="b1x2", name="b1x2")
        cw2 = cpool.tile([32, 32], f32, tag="cw2", name="cw2")
        b2x2 = cpool.tile([32, 1], f32, tag="b2x2", name="b2x2")
        rhs_gen = cpool.tile([33, GEN_W], bf16, tag="rhs_gen", name="rhs_gen")
        gammab = cpool.tile([64, 1], f32, tag="gammab", name="gammab")
        betab = cpool.tile([64, 1], f32, tag="betab", name="betab")
        maskc = cpool.tile([32, 2], f32, tag="maskc", name="maskc")
        fold128 = cpool.tile([128, 64], f32, tag="fold128", name="fold128")
        for t_, n_ in ((w1taps, "w1taps"), (b1x2, "b1x2"), (cw2, "cw2"),
                       (b2x2, "b2x2"), (gammab, "gammab"), (betab, "betab"),
                       (rhs_gen, "rhs_gen"), (maskc, "maskc"),
                       (fold128, "fold128")):
            nc.scalar.dma_start(t_[:], ins[n_])
        obs = [opool.tile([128, HW], bf16, tag=f"ob{p}", name=f"ob{p}")
               for p in range(NPAIR)]
        sts = [cpool.tile([128, 9 * 128], bf16, tag=f"st{p}", name=f"st{p}")
               for p in range(NPAIR)]
        sgens = [cpool.tile([33, 2], bf16, tag=f"sgen{p}", name=f"sgen{p}")
                 for p in range(NPAIR)]
        # zero the stationaries once (block structure is identical per pair);
        # set the all-ones bias row of the gen stationaries once
        for p in range(NPAIR):
            nc.gpsimd.memset(sts[p][:], 0.0)
            nc.gpsimd.memset(sgens[p][0:32, :], 0.0)
            nc.gpsimd.memset(sgens[p][32:33, :], 1.0)

        dbias2 = stpool.tile([128, NPAIR], bf16, tag="dbias2", name="dbias2")
        sums = stpool.tile([128, NPAIR * NT], f32, tag="sums", name="sums")
        sumsqs = stpool.tile([128, NPAIR * NT], f32, tag="sumsqs", name="sumsqs")

        def xview(p):
            # padded x: row h of the image lives at xv[:, h+1, 1:129]
            return xbs[p][:].rearrange("p (r w) -> p r w", r=H + 2, w=WP)

        def load_seg(p, ch):
            e0, e1 = WP * ROWSEG[ch], WP * ROWSEG[ch + 1]
            nc.sync.dma_start(xbs[p][:, e0:e1], x_d[2 * p:2 * p + 2, :, e0:e1])

        # E/Od row sums (even / odd columns); chunk ch covers image rows
        # 16ch..16ch+15 and only needs x segment ch // 2
        def eod_chunk(p, E, Od, ch):
            xv = xview(p)
            r0 = 1 + 16 * ch
            nc.vector.tensor_reduce(
                E[:, 16 * ch: 16 * ch + 16],
                xv[:, r0:r0 + 16, 1:128:2], axis=AX.X, op=ALU.add)
            nc.vector.tensor_reduce(
                Od[:, 16 * ch: 16 * ch + 16],
                xv[:, r0:r0 + 16, 2:129:2], axis=AX.X, op=ALU.add)

        def prep_pair(p, E, Od):
            """cond generator -> dynamic weights -> S_k stationary for pair p.

            E/Od row sums must already be computed (pipelined with the load).
            """
            xv = xview(p)
            # border columns 1, 127, 128 as f32 for the strided col sums
            bord = ppool.tile([128, 3 * H], f32, tag="bord", name="bord")
            colv = xv[:, 1:1 + H, :]
            nc.vector.tensor_copy(bord[:].rearrange("p (c r) -> p c r", c=3, r=H)[:, 0, :],
                                  colv[:, :, 127])
            nc.vector.tensor_copy(bord[:].rearrange("p (c r) -> p c r", c=3, r=H)[:, 1, :],
                                  colv[:, :, 128])
            nc.vector.tensor_copy(bord[:].rearrange("p (c r) -> p c r", c=3, r=H)[:, 2, :],
                                  colv[:, :, 1])
            # per-row strided col sums for kx = 0,1,2
            R = ppool.tile([128, 3 * H], f32, tag="R", name="R")
            nc.vector.tensor_tensor(
                out=R[:, 0:H], in0=E[:], in1=bord[:, 0:H], op=ALU.subtract)
            nc.vector.tensor_tensor(
                out=R[:, H:2 * H], in0=Od[:], in1=bord[:, H:2 * H], op=ALU.subtract)
            nc.vector.tensor_tensor(
                out=R[:, 2 * H:3 * H], in0=E[:], in1=bord[:, 2 * H:3 * H],
                op=ALU.subtract)
            # T[ci, k] strided row sums
            Tt = ppool.tile([128, 9], f32, tag="Tt", name="Tt")
            for k in range(9):
                ky, kx = k // 3, k % 3
                nc.vector.tensor_reduce(
                    Tt[:, k:k + 1],
                    R[:, kx * H + ky: kx * H + ky + 125: 2],  # 63 rows
                    axis=AX.X, op=ALU.add)

            # cond chain (tiny matmuls, plain fp32)
            pc1 = gpsum.tile([128, 512], f32, tag="gp", name="gp")
            for i, k in enumerate(range(9)):
                nc.tensor.matmul(
                    pc1[0:32, 0:1], w1taps[:, 32 * k: 32 * k + 32],
                    Tt[:, k:k + 1], start=(i == 0), stop=(i == 8))
            cond1 = ppool.tile([32, 1], f32, tag="cond1", name="cond1")
            nc.scalar.activation(cond1[:], pc1[0:32, 0:1], ACT.Relu, bias=b1x2[:])
            pc2 = gpsum.tile([128, 512], f32, tag="gp", name="gp")
            nc.tensor.matmul(pc2[0:32, 0:1], cw2[:], cond1[:])
            cond2 = ppool.tile([32, 1], f32, tag="cond2", name="cond2")
            nc.scalar.activation(cond2[:], pc2[0:32, 0:1], ACT.Relu, bias=b2x2[:])

            # gen stationary [33, 2]: col s = cond2_s (rows 16s..16s+16), row
            # 32 = 1 (preset).  sgen[i, s] = maskc[i, s] * cond2[i]
            sgen = sgens[p]
            nc.vector.tensor_scalar(
                out=sgen[0:32, :], in0=maskc[:], scalar1=cond2[:, 0:1],
                scalar2=None, op0=ALU.mult)

            # generator matmuls -> gen_sb [2, 4736] (pw | dw | dbias), relu'd
            gen_sb = ppool.tile([2, GEN_W], bf16, tag="gen_sb", name="gen_sb")
            for i in range(10):
                c0 = 512 * i
                n = min(512, GEN_W - c0)
                gp = gpsum.tile([128, 512], f32, tag="gp", name="gp")
                nc.tensor.matmul(
                    gp[0:2, 0:n], sgen[:], rhs_gen[:, c0:c0 + n])
                if c0 >= 4608:  # last chunk: dw tail (relu) + dbias (no relu)
                    nc.scalar.activation(gen_sb[0:2, 4608:4672], gp[0:2, 0:64], ACT.Relu)
                    nc.scalar.copy(gen_sb[0:2, 4672:4736], gp[0:2, 64:128])
                elif i % 2 == 0:
                    nc.scalar.activation(gen_sb[0:2, c0:c0 + n], gp[0:2, 0:n], ACT.Relu)
                else:
                    nc.vector.tensor_scalar(
                        out=gen_sb[0:2, c0:c0 + n], in0=gp[0:2, 0:n],
                        scalar1=0.0, scalar2=None, op0=ALU.max)

            # rearrange to channel-major layouts
            pwcb = ppool.tile([128, O], bf16, tag="pwcb", name="pwcb")
            dwcb_h = ppool.tile([128, 9], bf16, tag="dwcb_h", name="dwcb_h")
            for s in range(2):
                nc.sync.dma_start(
                    pwcb[s * 64:(s + 1) * 64, :],
                    gen_sb[s:s + 1, 0:O * C].rearrange(
                        "s (c o) -> s c o", o=O, c=C))
                nc.sync.dma_start(
                    dwcb_h[s * 64:(s + 1) * 64, :],
                    gen_sb[s:s + 1, O * C:O * C + C * 9].rearrange(
                        "s (c k) -> s c k", c=C, k=9))
                nc.sync.dma_start(dbias2[s * 64:(s + 1) * 64, p:p + 1],
                                  gen_sb[s:s + 1, 4672:4736])
            # f32 copy (scalar operands of tensor_scalar must be f32)
            dwcb = ppool.tile([128, 9], f32, tag="dwcb", name="dwcb")
            nc.vector.tensor_copy(dwcb[:], dwcb_h[:])

            # S_k stationary: S[s*64+c, 128k + s*64+o] = pw[o,c]*dw[c,k]
            # (k-outer order so conv tap k can start as soon as block k lands)
            st = sts[p]
            for k in range(9):
                for s in range(2):
                    nc.vector.tensor_scalar(
                        out=st[s * 64:(s + 1) * 64,
                               128 * k + s * 64: 128 * k + s * 64 + 64],
                        in0=pwcb[s * 64:(s + 1) * 64, :],
                        scalar1=dwcb[s * 64:(s + 1) * 64, k:k + 1],
                        scalar2=None, op0=ALU.mult)

        def conv_group(p, tiles):
            """conv tiles (4 output rows each) for pair p."""
            xv = xview(p)
            st = sts[p]
            ob = obs[p]
            psums = {}
            for t in tiles:
                psums[t] = cpsum.tile([128, 512], f32, tag="cp", name="cp")
            for i in range(9):
                ky, kx = i // 3, i % 3
                lhsT = st[:, 128 * i: 128 * i + 128]
                for t in tiles:
                    h0 = 4 * t
                    # out rows h0..h0+3, tap (ky,kx): x rows h0+ky-1.., cols +kx-1
                    nc.tensor.matmul(
                        psums[t][:],
                        lhsT,
                        xv[:, h0 + ky: h0 + ky + 4, kx:kx + 128],
                        start=(i == 0), stop=(i == 8))
            for t in tiles:
                col = NT * p + t
                # copy pre-BN conv tile into SBUF (bf16) + per-partition sum
                nc.vector.tensor_scalar(
                    out=ob[:, 512 * t: 512 * t + 512],
                    in0=psums[t][:], scalar1=0.0, scalar2=0.0, op0=ALU.add,
                    op1=ALU.add, accum_out=sums[:, col:col + 1])
                sq = sqpool.tile([128, 512], bf16, tag="sq", name="sq")
                nc.scalar.activation(
                    sq[:], psums[t][:], ACT.Square,
                    accum_out=sumsqs[:, col:col + 1])

        # per-pair stats fixup, overlapped with the other pair's conv:
        # produces [128, 2] (sum, sumsq) with the dbias fold applied
        sum2 = stpool.tile([128, NPAIR], f32, tag="sum2", name="sum2")
        ssq2 = stpool.tile([128, NPAIR], f32, tag="ssq2", name="ssq2")
        dbias2f = stpool.tile([128, NPAIR], f32, tag="dbias2f", name="dbias2f")

        def pair_stats(p):
            pc = p * NT
            nc.vector.tensor_copy(dbias2f[:, p:p + 1], dbias2[:, p:p + 1])
            nc.vector.tensor_reduce(
                sum2[:, p:p + 1], sums[:, pc:pc + NT], axis=AX.X, op=ALU.add)
            nc.vector.tensor_reduce(
                ssq2[:, p:p + 1], sumsqs[:, pc:pc + NT], axis=AX.X, op=ALU.add)
            d16k = stpool.tile([128, 1], f32, tag="d16k", name="d16k", bufs=2)
            nc.vector.tensor_scalar(out=d16k[:], in0=dbias2f[:, p:p + 1],
                                    scalar1=float(HW), scalar2=None, op0=ALU.mult)
            t1 = stpool.tile([128, 1], f32, tag="t1", name="t1", bufs=2)
            # t1 = 2*d*sum + n*d^2 = d*(2*sum + n*d)
            nc.vector.tensor_scalar(out=t1[:], in0=sum2[:, p:p + 1], scalar1=2.0,
                                    scalar2=None, op0=ALU.mult)
            nc.vector.tensor_tensor(out=t1[:], in0=t1[:], in1=d16k[:], op=ALU.add)
            nc.vector.tensor_tensor(out=t1[:], in0=t1[:], in1=dbias2f[:, p:p + 1],
                                    op=ALU.mult)
            nc.vector.tensor_tensor(out=ssq2[:, p:p + 1], in0=ssq2[:, p:p + 1],
                                    in1=t1[:], op=ALU.add)
            nc.vector.tensor_tensor(out=sum2[:, p:p + 1], in0=sum2[:, p:p + 1],
                                    in1=d16k[:], op=ALU.add)

        # ---------------- warmups ----------------
        # dummy collective to warm the CC stream / gpsimd path while the x
        # loads run, and activation-table preloads (Sqrt/Identity are needed
        # on the post-collective critical path)
        wsb = stpool.tile([64, 2], f32, tag="wsb", name="wsb")
        nc.gpsimd.memset(wsb[:], 0.0)
        wu_in = dpool.tile([64, 2], f32, tag="wu_in", name="wu_in")
        wu_out = dpool.tile([64, 16], f32, tag="wu_out", name="wu_out")
        nc.sync.dma_start(wu_in[:], wsb[:])
        nc.gpsimd.collective_compute(
            "AllGather", ALU.bypass,
            replica_groups=[list(range(NCORES))],
            ins=[wu_in[:].opt()], outs=[wu_out[:].opt()])
        dumt = stpool.tile([64, 2], f32, tag="dumt", name="dumt")
        nc.scalar.activation(dumt[:, 0:1], gammab[:], ACT.Sqrt)
        nc.scalar.activation(dumt[:, 1:2], gammab[:], ACT.Identity)

        # ---------------- main schedule ----------------
        # pair 0: row sums pipelined behind the (already issued) load
        E0 = ppool.tile([128, H], f32, tag="E", name="E0")
        Od0 = ppool.tile([128, H], f32, tag="Od", name="Od0")
        for ch in range(8):
            eod_chunk(0, E0, Od0, ch)
        prep_pair(0, E0, Od0)

        E1 = ppool.tile([128, H], f32, tag="E", name="E1")
        Od1 = ppool.tile([128, H], f32, tag="Od", name="Od1")
        for ch in range(NSEG):
            load_seg(1, ch)

        # 3-tile conv groups double-buffer perfectly in the 6 PSUM banks; the
        # short last group keeps the end-of-conv stats drain small
        groups = [list(range(3 * g, 3 * g + 3)) for g in range(10)] + [[30, 31]]
        # pair-1 row sums + prep interleaved early behind pair-0 groups so the
        # whole second half of the conv phase is uninterrupted
        for g, tiles in enumerate(groups):
            conv_group(0, tiles)
            if g < 4:
                eod_chunk(1, E1, Od1, 2 * g)
                eod_chunk(1, E1, Od1, 2 * g + 1)
            elif g == 4:
                prep_pair(1, E1, Od1)
        pair_stats(0)
        for tiles in groups:
            conv_group(1, tiles)
        pair_stats(1)

        # ---------------- BN statistics ----------------
        st128 = stpool.tile([128, 2], f32, tag="st128", name="st128")
        nc.vector.tensor_reduce(st128[:, 0:1], sum2[:], axis=AX.X, op=ALU.add)
        nc.vector.tensor_reduce(st128[:, 1:2], ssq2[:], axis=AX.X, op=ALU.add)
        # fold the two sample-halves [128,2] -> [64,2] on the (idle) PE
        pfold = gpsum.tile([128, 512], f32, tag="gp", name="gp")
        nc.tensor.matmul(pfold[0:64, 0:2], fold128[:], st128[:])
        st64 = stpool.tile([64, 2], f32, tag="st64", name="st64")
        nc.vector.tensor_copy(st64[:], pfold[0:64, 0:2])

        # all-gather across the 8 cores via DRAM bounce buffers, reduced
        # locally (gather is a single phase; cheaper than Mesh AllReduce)
        cc_in = dpool.tile([64, 2], f32, tag="cc_in", name="cc_in")
        # gather output is a flat concat of the 8 cores' 512B buffers
        cc_out = dpool.tile([NCORES, 64, 2], f32, tag="cc_out", name="cc_out")
        nc.sync.dma_start(cc_in[:], st64[:])
        nc.gpsimd.collective_compute(
            "AllGather", ALU.bypass,
            replica_groups=[list(range(NCORES))],
            ins=[cc_in[:].opt()], outs=[cc_out[:].opt()])
        ccsb = stpool.tile([64, 2 * NCORES], f32, tag="ccsb", name="ccsb")
        for r in range(NCORES):
            eng = nc.sync if r % 2 == 0 else nc.scalar
            eng.dma_start(ccsb[:, 2 * r:2 * r + 2], cc_out[r])
        stg = stpool.tile([128, 2], f32, tag="stg", name="stg")
        nc.vector.tensor_reduce(
            stg[0:64, :], ccsb[:].rearrange("p (r c) -> p c r", r=NCORES, c=2),
            axis=AX.X, op=ALU.add)
        nc.sync.dma_start(stg[64:128, :], stg[0:64, :])

        # scale/shift: S = gamma/sqrt(var+eps) ; T2[:,p] = dbias*S + (beta - mean*S)
        gb2 = stpool.tile([128, 1], f32, tag="gb2", name="gb2")
        bb2 = stpool.tile([128, 1], f32, tag="bb2", name="bb2")
        nc.sync.dma_start(gb2[0:64, :], ins["gammab"])
        nc.sync.dma_start(gb2[64:128, :], ins["gammab"])
        nc.sync.dma_start(bb2[0:64, :], ins["betab"])
        nc.sync.dma_start(bb2[64:128, :], ins["betab"])

        ntot = float(BPC * NCORES * HW)
        ms = stpool.tile([128, 2], f32, tag="ms", name="ms")
        nc.vector.tensor_scalar(out=ms[:], in0=stg[:], scalar1=1.0 / ntot,
                                scalar2=None, op0=ALU.mult)  # (mean, E[x^2])
        var = stpool.tile([128, 1], f32, tag="var", name="var")
        nc.vector.tensor_tensor(out=var[:], in0=ms[:, 0:1], in1=ms[:, 0:1],
                                op=ALU.mult)
        nc.vector.tensor_tensor(out=var[:], in0=ms[:, 1:2], in1=var[:],
                                op=ALU.subtract)
        nc.vector.tensor_scalar(out=var[:], in0=var[:], scalar1=EPS,
                                scalar2=None, op0=ALU.add)
        std = stpool.tile([128, 1], f32, tag="std", name="std")
        nc.scalar.activation(std[:], var[:], ACT.Sqrt)
        inv = stpool.tile([128, 1], f32, tag="inv", name="inv")
        nc.vector.reciprocal(inv[:], std[:])
        Sb = stpool.tile([128, 1], f32, tag="Sb", name="Sb")
        nc.vector.tensor_tensor(out=Sb[:], in0=inv[:], in1=gb2[:], op=ALU.mult)
        Tb = stpool.tile([128, 1], f32, tag="Tb", name="Tb")
        nc.vector.tensor_tensor(out=Tb[:], in0=ms[:, 0:1], in1=Sb[:], op=ALU.mult)
        nc.vector.tensor_tensor(out=Tb[:], in0=bb2[:], in1=Tb[:], op=ALU.subtract)
        T2 = stpool.tile([128, NPAIR], f32, tag="T2", name="T2")
        nc.vector.tensor_scalar(out=T2[:], in0=dbias2f[:], scalar1=Sb[:],
                                scalar2=Tb[:], op0=ALU.mult, op1=ALU.add)

        # ---------------- final affine + store ----------------
        # alternate chunks between Vector and Scalar so the two engines share
        # the affine while the store DMA is the bottleneck
        CH = 2048
        for i in range(HW // CH):
            c0 = CH * i
            for p in range(NPAIR):
                ob = obs[p]
                if (i + p) % 2 == 0:
                    nc.vector.tensor_scalar(
                        out=ob[:, c0:c0 + CH], in0=ob[:, c0:c0 + CH],
                        scalar1=Sb[:], scalar2=T2[:, p:p + 1],
                        op0=ALU.mult, op1=ALU.add)
                else:
                    nc.scalar.activation(
                        ob[:, c0:c0 + CH], ob[:, c0:c0 + CH], ACT.Identity,
                        bias=T2[:, p:p + 1], scale=Sb[:])
                # alternate issue queues so DMA issue isn't serialized
                deng = nc.sync if (i + p) % 2 == 0 else nc.scalar
                deng.dma_start(
                    y_d[2 * p:2 * p + 2, :, c0:c0 + CH], ob[:, c0:c0 + CH])


# ---------------------------------------------------------------------------
# build + run
# ---------------------------------------------------------------------------
_CACHE = {}


def _build():
    if "nc" in _CACHE:
        return _CACHE["nc"]
    from concourse import bacc, mybir, tile

    nc = bacc.Bacc("TRN2", target_bir_lowering=False, debug=False,
                   num_devices=NCORES)
    f32 = mybir.dt.float32
    bf16 = mybir.dt.bfloat16
    ins = {
        "x": nc.dram_tensor("x", [BPC, C, H + 2, W + 2], bf16, kind="ExternalInput").ap(),
        "w1taps": nc.dram_tensor("w1taps", [128, 9 * 32], f32, kind="ExternalInput").ap(),
        "b1x2": nc.dram_tensor("b1x2", [32, 1], f32, kind="ExternalInput").ap(),
        "cw2": nc.dram_tensor("cw2", [32, 32], f32, kind="ExternalInput").ap(),
        "b2x2": nc.dram_tensor("b2x2", [32, 1], f32, kind="ExternalInput").ap(),
        "rhs_gen": nc.dram_tensor("rhs_gen", [33, GEN_W], bf16, kind="ExternalInput").ap(),
        "gammab": nc.dram_tensor("gammab", [64, 1], f32, kind="ExternalInput").ap(),
        "betab": nc.dram_tensor("betab", [64, 1], f32, kind="ExternalInput").ap(),
        "maskc": nc.dram_tensor("maskc", [32, 2], f32, kind="ExternalInput").ap(),
        "fold128": nc.dram_tensor("fold128", [128, 64], f32, kind="ExternalInput").ap(),
    }
    outs = {"y": nc.dram_tensor("y", [BPC, C, H, W], bf16, kind="ExternalOutput").ap()}
    with tile.TileContext(nc) as tc:
        body(tc, outs, ins)
    nc.compile()
    _CACHE["nc"] = nc
    return nc


def make_in_maps(inputs):
    x = np.asarray(inputs["x"], np.float32)
    xp = np.zeros((B, C, H + 2, W + 2), BF16)
    xp[:, :, 1:H + 1, 1:W + 1] = x.astype(BF16)
    consts = _prep_consts(inputs)
    in_maps = []
    for c in range(NCORES):
        m = {"x": np.ascontiguousarray(xp[BPC * c: BPC * (c + 1)])}
        m.update(consts)
        in_maps.append(m)
    return in_maps


def run(inputs, trace=False):
    from concourse.bass_utils import run_bass_kernel_spmd

    nc = _build()
    in_maps = make_in_maps(inputs)
    res = run_bass_kernel_spmd(nc, in_maps, core_ids=list(range(NCORES)),
                               trace=trace)
    y = np.concatenate(
        [np.asarray(res.results[c]["y"]).astype(np.float32)
         for c in range(NCORES)], axis=0)
    return y, res


def kernel(**inputs) -> np.ndarray:
    y, _ = run(inputs, trace=False)
    return y
